# revision 1
# baseline (speedup 1.0000x reference)
"""Trainium2 Bass kernel for nn_GatherModel (NNConv GNN message passing).

8-core SPMD, edge-parallel sharded by destination node block:
  - core k owns nodes [k*6250, (k+1)*6250) and all edges whose dst lies there
  - per-edge weight matrices W'_e (o-major) are built once on device (PE) and
    streamed from HBM each of the 6 message-passing steps
  - per-edge contraction msg = x_src @ W_e runs on the Vector engine via a
    custom fused multiply+prefix-scan DVE op, extracting per-o sums by
    strided prefix differences
  - scatter (segment-sum over dst) is a PE matmul against on-device-built
    one-hot window matrices; node update runs in transposed feature layout
  - each step ends with an 8-core AllGather of the updated node features
"""
import numpy as np

import concourse.bacc as bacc
import concourse.bass as bass
import concourse.mybir as mybir
import concourse.tile as tile
from concourse import bass_utils, dve_ops
from concourse.dve_spec import Spec, Src0, Src1, scan, AluOp, lower, _has_src1
from concourse.dve_uop import DveOpSpec

N = 50000
E = 150000
D_IN = 42
D_H = 42
E_IN = 10
E_H = 128
STEPS = 6
N_CORES = 8
NPC = N // N_CORES          # 6250 nodes per core
WIN = 128                   # scatter window (node block) size
N_WIN = (NPC + WIN - 1) // WIN  # 49 windows per core, last partial (106)
NW = D_H * D_H              # 1764
F32 = mybir.dt.float32
I32 = mybir.dt.int32


def _register_prefix_mac():
    name = "PREFIX_MAC_GNN"
    if name in dve_ops._SUB_OPCODE_FOR_NAME:
        return next(op for op in dve_ops.OPS if op.name == name)
    spec = Spec(
        body=scan(AluOp.ADD, Src0 * Src1),
        reference=lambda in0, in1, s0, s1, imm2: np.cumsum(
            (in0.astype(np.float32) * in1).reshape(in0.shape[0], -1), axis=-1
        ),
    )
    shas = {}
    row = dve_ops._CUSTOM_DVE_ROW_BASE + len(dve_ops.OPS)
    for ver in ("v3", "v4"):
        uops = lower(spec, ver=ver)
        shas[ver] = DveOpSpec(name=name, opcode=row, uops=uops,
                              rd1_en=_has_src1(spec)).sha(ver)
    op = dve_ops.DveOp(name, spec, subdim=False, uops_sha=shas)
    dve_ops.OPS.append(op)
    dve_ops._SUB_OPCODE_FOR_NAME[name] = row
    dve_ops.CUSTOM_DVE_SPECS[name] = spec
    return op


def _host_prep(n_feat, e_feat, src, dst):
    """Sort edges by dst, shard by dst block, pad each (core, window) edge run
    onto a shared slot grid so the tile->window map is identical on all cores."""
    order = np.argsort(dst, kind="stable")
    src_s, dst_s, ef_s = src[order], dst[order], e_feat[order]

    # per (core, window) counts
    core_e = dst_s // NPC
    loc = dst_s - core_e * NPC
    win_e = loc // WIN
    cnt = np.zeros((N_CORES, N_WIN), dtype=np.int64)
    np.add.at(cnt, (core_e, win_e), 1)

    slot_cnt = cnt.max(axis=0)                       # shared grid
    G = np.concatenate([[0], np.cumsum(slot_cnt)])   # window slot boundaries
    total = int(G[-1])
    T = (total + 127) // 128                         # edge tiles per core
    E_PAD = T * 128

    # per-core padded edge arrays
    src_pad = np.zeros((N_CORES, E_PAD), dtype=np.int32)
    dstrel_pad = np.full((N_CORES, E_PAD), -1.0, dtype=np.float32)
    ef_pad = np.zeros((N_CORES, E_PAD, E_IN), dtype=np.float32)

    # tile -> window band
    w0 = np.zeros(T, dtype=np.int64)       # first window overlapping tile t
    bw = np.zeros(T, dtype=np.int64)       # how many windows overlap tile t
    for t in range(T):
        lo, hi = t * 128, min((t + 1) * 128, total)
        wlo = int(np.searchsorted(G, lo, side="right") - 1)
        whi = int(np.searchsorted(G, max(hi - 1, lo), side="right") - 1)
        wlo, whi = min(wlo, N_WIN - 1), min(whi, N_WIN - 1)
        w0[t] = wlo
        bw[t] = whi - wlo + 1
    B_W = int(bw.max())

    # fill padded arrays: window w of core k occupies slots [G[w], G[w]+cnt[k,w])
    core_starts = np.searchsorted(core_e, np.arange(N_CORES))
    for k in range(N_CORES):
        base = core_starts[k]
        cw = np.concatenate([[0], np.cumsum(cnt[k])])
        for w in range(N_WIN):
            s0, s1 = int(base + cw[w]), int(base + cw[w + 1])
            g0 = int(G[w])
            n_e = s1 - s0
            src_pad[k, g0:g0 + n_e] = src_s[s0:s1]
            ef_pad[k, g0:g0 + n_e] = ef_s[s0:s1]
            # dst_rel relative to the band anchor of the edge's tile
            slots = np.arange(g0, g0 + n_e)
            dstrel_pad[k, g0:g0 + n_e] = (
                loc[s0:s1] - w0[slots // 128] * WIN).astype(np.float32)

    # scatter pair list (t, w) from actual overlap, and per-window tile ranges
    pairs = []
    for t in range(T):
        for j in range(int(bw[t])):
            w = int(w0[t]) + j
            if w < N_WIN:
                pairs.append((t, w))
    win_tiles = {w: [t for (t, ww) in pairs if ww == w] for w in range(N_WIN)}

    grid = dict(T=T, E_PAD=E_PAD, B_W=B_W, w0=w0, bw=bw, win_tiles=win_tiles)

    per_core = []
    for k in range(N_CORES):
        per_core.append(dict(
            e_featT=np.ascontiguousarray(ef_pad[k].T),           # [10, E_PAD]
            n_featT=np.ascontiguousarray(n_feat[k * NPC:(k + 1) * NPC].T),  # [42, NPC]
            src_idx=np.ascontiguousarray(src_pad[k].reshape(T, 128).T).astype(np.int32),  # [128, T]
            dst_rel=np.ascontiguousarray(dstrel_pad[k].reshape(T, 128).T),  # [128, T]
        ))
    return grid, per_core


def _build_program(grid):
    T, B_W = grid["T"], grid["B_W"]
    w0, bw, win_tiles = grid["w0"], grid["bw"], grid["win_tiles"]
    PREFIX_MAC = _register_prefix_mac()

    nc = bacc.Bacc("TRN2", target_bir_lowering=False, debug=False,
                   num_devices=N_CORES)

    # ---- kernel I/O ----
    e_featT = nc.dram_tensor("e_featT", [E_IN, grid["E_PAD"]], F32, kind="ExternalInput")
    n_featT = nc.dram_tensor("n_featT", [D_IN, NPC], F32, kind="ExternalInput")
    src_idx = nc.dram_tensor("src_idx", [128, T], I32, kind="ExternalInput")
    dst_rel = nc.dram_tensor("dst_rel", [128, T], F32, kind="ExternalInput")
    iota = nc.dram_tensor("iota", [128, B_W * WIN], F32, kind="ExternalInput")
    en1_w = nc.dram_tensor("en1_w", [E_IN, E_H], F32, kind="ExternalInput")
    en1_b = nc.dram_tensor("en1_b", [1, E_H], F32, kind="ExternalInput")
    en2_wp = nc.dram_tensor("en2_wp", [E_H, NW], F32, kind="ExternalInput")
    en2_bp = nc.dram_tensor("en2_bp", [1, NW], F32, kind="ExternalInput")
    b_r = nc.dram_tensor("b_r", [D_H, D_H], F32, kind="ExternalInput")
    lin0_wt = nc.dram_tensor("lin0_wt", [D_IN, D_H], F32, kind="ExternalInput")
    lin0_br = nc.dram_tensor("lin0_br", [1, D_H], F32, kind="ExternalInput")
    msgw_top = nc.dram_tensor("msgw_top", [D_H, D_H], F32, kind="ExternalInput")
    msgw_bot = nc.dram_tensor("msgw_bot", [D_H, D_H], F32, kind="ExternalInput")
    msgb_r = nc.dram_tensor("msgb_r", [1, D_H], F32, kind="ExternalInput")
    convb_r = nc.dram_tensor("convb_r", [1, D_H], F32, kind="ExternalInput")
    ident = nc.dram_tensor("ident", [D_H, D_H], F32, kind="ExternalInput")
    ones_r = nc.dram_tensor("ones_r", [1, 128], F32, kind="ExternalInput")
    y = nc.dram_tensor("y", [NPC, D_H], F32, kind="ExternalOutput")

    with tile.TileContext(nc) as tc:
        with (
            tc.tile_pool(name="const", bufs=1) as cpool,
            tc.tile_pool(name="dram", bufs=1, space="DRAM") as dram,
        ):
            # ---- persistent SBUF residents ----
            nfT_sb = cpool.tile([D_IN, NPC], F32)
            srci_sb = cpool.tile([128, T], I32)
            dstr_sb = cpool.tile([128, T], F32)
            iota_sb = cpool.tile([128, B_W * WIN], F32)
            en1w_sb = cpool.tile([E_IN, E_H], F32)
            en1b_sb = cpool.tile([1, E_H], F32)
            en2wp_sb = cpool.tile([E_H, NW], F32)
            en2bp_sb = cpool.tile([1, NW], F32)
            br_sb = cpool.tile([D_H, D_H], F32)
            lin0w_sb = cpool.tile([D_IN, D_H], F32)
            lin0b_sb = cpool.tile([1, D_H], F32)
            mwt_sb = cpool.tile([D_H, D_H], F32)
            mwb_sb = cpool.tile([D_H, D_H], F32)
            mb_sb = cpool.tile([1, D_H], F32)
            cvb_sb = cpool.tile([1, D_H], F32)
            id_sb = cpool.tile([D_H, D_H], F32)
            ones_sb = cpool.tile([1, 128], F32)
            outT_a = cpool.tile([D_H, NPC], F32)
            outT_b = cpool.tile([D_H, NPC], F32)
            pfx = cpool.tile([128, 1 + NW], F32)

            for sb, dr in [(nfT_sb, n_featT), (srci_sb, src_idx),
                           (dstr_sb, dst_rel), (iota_sb, iota), (en1w_sb, en1_w),
                           (en1b_sb, en1_b), (en2wp_sb, en2_wp), (en2bp_sb, en2_bp), (br_sb, b_r),
                           (lin0w_sb, lin0_wt), (lin0b_sb, lin0_br), (mwt_sb, msgw_top),
                           (mwb_sb, msgw_bot), (mb_sb, msgb_r), (cvb_sb, convb_r),
                           (id_sb, ident), (ones_sb, ones_r)]:
                nc.sync.dma_start(sb[:], dr[:])
            nc.gpsimd.memset(pfx[:, 0:1], 0.0)

            # ---- DRAM scratch ----
            w_dram = dram.tile([T * 128, NW], F32)
            cc_in = [dram.tile([NPC, D_H], F32, name=f"cc_in{i}") for i in range(2)]
            cc_out = [dram.tile([N, D_H], F32, name=f"cc_out{i}", addr_space="Shared")
                      for i in range(STEPS)]

            # =========== setup: build W' in HBM ===========
            ECH = 16  # e_feat tiles per SBUF chunk
            with (
                tc.tile_pool(name="su_sb", bufs=3) as su_sb,
                tc.tile_pool(name="su_e", bufs=2) as su_e,
                tc.tile_pool(name="su_bias", bufs=1) as su_bias,
                tc.tile_pool(name="su_ph", bufs=2, space="PSUM") as su_ph,
                tc.tile_pool(name="su_pw", bufs=1, space="PSUM") as su_pw,
            ):
                pw = [su_pw.tile([128, 512], F32, name=f"pw{j}") for j in range(4)]
                nsz = [512, 512, 512, NW - 3 * 512]
                # broadcast en2 bias across all 128 partitions, once
                bias_sb = su_bias.tile([128, NW], F32)
                for j in range(4):
                    o0 = j * 512
                    nc.tensor.matmul(pw[j][:, :nsz[j]], lhsT=ones_sb[:1, :],
                                     rhs=en2bp_sb[:, o0:o0 + nsz[j]],
                                     start=True, stop=True)
                    nc.scalar.copy(bias_sb[:, o0:o0 + nsz[j]], pw[j][:, :nsz[j]])
                e_ch = None
                for t in range(T):
                    if t % ECH == 0:
                        c0 = t * 128
                        c1 = min((t + ECH) * 128, grid["E_PAD"])
                        e_ch = su_e.tile([E_IN, ECH * 128], F32, name="e_ch")
                        nc.sync.dma_start(e_ch[:, :c1 - c0], e_featT[:, c0:c1])
                    ph = su_ph.tile([128, 128], F32, name="ph")
                    o = (t % ECH) * 128
                    nc.tensor.matmul(ph[:], lhsT=en1w_sb[:], rhs=e_ch[:, o:o + 128],
                                     start=True, stop=False)
                    nc.tensor.matmul(ph[:], lhsT=en1b_sb[:], rhs=ones_sb[:1, :],
                                     start=False, stop=True)
                    h_sb = su_sb.tile([128, 128], F32, name="h_sb")
                    nc.scalar.activation(h_sb[:], ph[:], mybir.ActivationFunctionType.Relu)
                    w_sb = su_sb.tile([128, NW], F32, name="w_sb")
                    for j in range(4):
                        o0 = j * 512
                        nc.tensor.matmul(pw[j][:, :nsz[j]], lhsT=h_sb[:],
                                         rhs=en2wp_sb[:, o0:o0 + nsz[j]],
                                         start=True, stop=True)
                        nc.vector.tensor_tensor(
                            out=w_sb[:, o0:o0 + nsz[j]], in0=pw[j][:, :nsz[j]],
                            in1=bias_sb[:, o0:o0 + nsz[j]], op=mybir.AluOpType.add)
                    nc.sync.dma_start(w_dram[t * 128:(t + 1) * 128, :], w_sb[:])

            # =========== step pools ===========
            with (
                tc.tile_pool(name="st_w", bufs=12) as p_w,
                tc.tile_pool(name="st_x", bufs=6) as p_x,
                tc.tile_pool(name="st_m", bufs=4) as p_m,
                tc.tile_pool(name="st_oh", bufs=4) as p_oh,
                tc.tile_pool(name="st_sm", bufs=3) as p_sm,
                tc.tile_pool(name="ps_ag", bufs=3, space="PSUM") as ps_ag,
                tc.tile_pool(name="ps_st", bufs=2, space="PSUM") as ps_st,
                tc.tile_pool(name="ps_up", bufs=2, space="PSUM") as ps_up,
                tc.tile_pool(name="ps_tr", bufs=1, space="PSUM") as ps_tr,
            ):
                def window_cols(w):
                    n0 = w * WIN
                    m = min(WIN, NPC - n0)
                    return n0, m

                def update_window(w, outT_cur, outT_new, aggr_ps, step):
                    """Window epilogue: finish aggr, relu, update matmul, transpose, DMA."""
                    n0, m = window_cols(w)
                    last = step == STEPS
                    # + out (identity residual into conv) and conv bias
                    nc.tensor.matmul(aggr_ps[:, :m], lhsT=id_sb[:],
                                     rhs=outT_cur[:, n0:n0 + m], start=False, stop=False)
                    nc.tensor.matmul(aggr_ps[:, :m], lhsT=cvb_sb[:],
                                     rhs=ones_sb[:1, :m], start=False, stop=True)
                    mT_sb = p_sm.tile([D_H, WIN], F32, name="mT_sb")
                    nc.scalar.activation(mT_sb[:, :m], aggr_ps[:, :m],
                                         mybir.ActivationFunctionType.Relu)
                    up = ps_up.tile([D_H, WIN], F32, name="up")
                    nc.tensor.matmul(up[:, :m], lhsT=mwt_sb[:], rhs=mT_sb[:, :m],
                                     start=True, stop=False)
                    nc.tensor.matmul(up[:, :m], lhsT=mwb_sb[:], rhs=outT_cur[:, n0:n0 + m],
                                     start=False, stop=False)
                    nc.tensor.matmul(up[:, :m], lhsT=mb_sb[:], rhs=ones_sb[:1, :m],
                                     start=False, stop=not last)
                    if last:
                        nc.tensor.matmul(up[:, :m], lhsT=id_sb[:], rhs=nfT_sb[:, n0:n0 + m],
                                         start=False, stop=True)
                    nc.scalar.copy(outT_new[:, n0:n0 + m], up[:, :m])
                    tr = ps_tr.tile([128, D_H], F32, name="tr")
                    nc.tensor.transpose(tr[:m, :], outT_new[:, n0:n0 + m], id_sb[:])
                    rows = p_sm.tile([128, D_H], F32, name="rows")
                    nc.scalar.copy(rows[:m, :], tr[:m, :])
                    if last:
                        nc.sync.dma_start(y[n0:n0 + m, :], rows[:m, :])
                    else:
                        nc.sync.dma_start(cc_in[step % 2][n0:n0 + m, :], rows[:m, :])

                def all_gather(step):
                    nc.gpsimd.collective_compute(
                        "AllGather", mybir.AluOpType.bypass,
                        replica_groups=[list(range(N_CORES))],
                        ins=[cc_in[step % 2].opt()], outs=[cc_out[step].opt()])

                # =========== lin0: out0 = relu(n_feat @ lin0_w + b) ===========
                for w in range(N_WIN):
                    n0, m = window_cols(w)
                    up = ps_up.tile([D_H, WIN], F32, name="up")
                    nc.tensor.matmul(up[:, :m], lhsT=lin0w_sb[:], rhs=nfT_sb[:, n0:n0 + m],
                                     start=True, stop=False)
                    nc.tensor.matmul(up[:, :m], lhsT=lin0b_sb[:], rhs=ones_sb[:1, :m],
                                     start=False, stop=True)
                    nc.scalar.activation(outT_a[:, n0:n0 + m], up[:, :m],
                                         mybir.ActivationFunctionType.Relu)
                    tr = ps_tr.tile([128, D_H], F32, name="tr")
                    nc.tensor.transpose(tr[:m, :], outT_a[:, n0:n0 + m], id_sb[:])
                    rows = p_sm.tile([128, D_H], F32, name="rows")
                    nc.scalar.copy(rows[:m, :], tr[:m, :])
                    nc.sync.dma_start(cc_in[0][n0:n0 + m, :], rows[:m, :])
                all_gather(0)

                # =========== message passing steps ===========
                for step in range(1, STEPS + 1):
                    outT_cur = outT_a if step % 2 == 1 else outT_b
                    outT_new = outT_b if step % 2 == 1 else outT_a
                    src_buf = cc_out[step - 1]
                    aggr_of = {}
                    for t in range(T):
                        x_g = p_x.tile([128, D_H], F32, name="x_g")
                        nc.gpsimd.indirect_dma_start(
                            out=x_g[:], out_offset=None, in_=src_buf[:],
                            in_offset=bass.IndirectOffsetOnAxis(
                                ap=srci_sb[:, t:t + 1], axis=0))
                        w_t = p_w.tile([128, NW], F32, name="w_t")
                        nc.sync.dma_start(w_t[:], w_dram[t * 128:(t + 1) * 128, :])
                        nc.vector._custom_dve(
                            PREFIX_MAC, out=pfx[:, 1:1 + NW], in0=w_t[:],
                            in1=x_g[:, None, :].to_broadcast([128, D_H, D_H]))
                        msg = p_m.tile([128, D_H], F32, name="msg")
                        nc.vector.tensor_tensor(
                            out=msg[:], in0=pfx[:, D_H:1 + NW:D_H],
                            in1=pfx[:, 0:NW:D_H], op=mybir.AluOpType.subtract)
                        bwt = int(bw[t])
                        oh = p_oh.tile([128, B_W * WIN], F32, name="oh")
                        nc.vector.tensor_scalar(
                            out=oh[:, :bwt * WIN], in0=iota_sb[:, :bwt * WIN],
                            scalar1=dstr_sb[:, t:t + 1],
                            scalar2=None, op0=mybir.AluOpType.is_equal)
                        # scatter matmuls
                        for j in range(bwt):
                            w = int(w0[t]) + j
                            if w >= N_WIN:
                                continue
                            tiles_w = win_tiles[w]
                            if w not in aggr_of:
                                aggr_of[w] = ps_ag.tile([D_H, WIN], F32, name="aggr")
                            first = t == tiles_w[0]
                            last_t = t == tiles_w[-1]
                            nc.tensor.matmul(aggr_of[w][:], lhsT=msg[:],
                                             rhs=oh[:, j * WIN:(j + 1) * WIN],
                                             start=first, stop=False)
                            if last_t:
                                update_window(w, outT_cur, outT_new,
                                              aggr_of.pop(w), step)
                    if step < STEPS:
                        all_gather(step)

    nc.compile()
    return nc


_CACHED = {}


def kernel(n_feat, e_feat, src, dst, lin0_w, lin0_b, en1_w, en1_b,
           en2_w, en2_b, conv_bias, msg_w, msg_b):
    n_feat = np.asarray(n_feat, dtype=np.float32)
    e_feat = np.asarray(e_feat, dtype=np.float32)
    src = np.asarray(src, dtype=np.int32)
    dst = np.asarray(dst, dtype=np.int32)

    grid, per_core = _host_prep(n_feat, e_feat, src, dst)

    key = (grid["T"], grid["B_W"], tuple(grid["w0"].tolist()))
    if key not in _CACHED:
        _CACHED.clear()
        _CACHED[key] = _build_program(grid)
    nc = _CACHED[key]

    en2_wp = np.ascontiguousarray(
        np.asarray(en2_w, np.float32).reshape(E_H, D_H, D_H).transpose(0, 2, 1).reshape(E_H, NW))
    shared = dict(
        iota=np.tile(np.arange(grid["B_W"] * WIN, dtype=np.float32), (128, 1)),
        en1_w=np.asarray(en1_w, np.float32),
        en1_b=np.asarray(en1_b, np.float32).reshape(1, E_H),
        en2_wp=en2_wp,
        en2_bp=np.ascontiguousarray(
            np.asarray(en2_b, np.float32).reshape(D_H, D_H).T.reshape(1, NW)),
        b_r=np.ascontiguousarray(np.asarray(en2_b, np.float32).reshape(D_H, D_H)),
        lin0_wt=np.asarray(lin0_w, np.float32),
        lin0_br=np.asarray(lin0_b, np.float32).reshape(1, D_H),
        msgw_top=np.ascontiguousarray(np.asarray(msg_w, np.float32)[:D_H, :]),
        msgw_bot=np.ascontiguousarray(np.asarray(msg_w, np.float32)[D_H:, :]),
        msgb_r=np.asarray(msg_b, np.float32).reshape(1, D_H),
        convb_r=np.asarray(conv_bias, np.float32).reshape(1, D_H),
        ident=np.eye(D_H, dtype=np.float32),
        ones_r=np.ones((1, 128), dtype=np.float32),
    )
    in_maps = []
    for k in range(N_CORES):
        m = dict(shared)
        m.update(per_core[k])
        in_maps.append(m)

    res = bass_utils.run_bass_kernel_spmd(nc, in_maps, core_ids=list(range(N_CORES)))
    out = np.concatenate([res.results[k]["y"] for k in range(N_CORES)], axis=0)
    return out.astype(np.float32)



# revision 3
# speedup vs baseline: 1.3381x; 1.3381x over previous
"""Trainium2 Bass kernel for nn_GatherModel (NNConv GNN message passing).

8-core SPMD, edge-parallel sharded by destination node block:
  - core k owns nodes [k*6250, (k+1)*6250) and all edges whose dst lies there
  - per-edge weight matrices W'_e (o-major) are built once on device (PE,
    bf16) and streamed bf16 from HBM each of the 6 message-passing steps
  - per-edge contraction msg = x_src @ W_e runs on the Vector engine via a
    custom fused multiply+prefix-scan DVE op (bf16 in, fp32 out),
    extracting per-o sums by strided prefix differences
  - scatter (segment-sum over dst) is a PE matmul against on-device-built
    bf16 one-hot window matrices; node update runs fp32 in transposed
    feature layout
  - each step ends with an 8-core AllGather of bf16 node features
"""
import numpy as np
import ml_dtypes

import concourse.bacc as bacc
import concourse.bass as bass
import concourse.mybir as mybir
import concourse.tile as tile
from concourse import bass_utils, dve_ops
from concourse.dve_spec import Spec, Src0, Src1, scan, AluOp, lower, _has_src1
from concourse.dve_uop import DveOpSpec

N = 50000
E = 150000
D_IN = 42
D_H = 42
E_IN = 10
E_H = 128
STEPS = 6
N_CORES = 8
NPC = N // N_CORES          # 6250 nodes per core
WIN = 128                   # scatter window (node block) size
N_WIN = (NPC + WIN - 1) // WIN  # 49 windows per core, last partial (106)
NW = D_H * D_H              # 1764
F32 = mybir.dt.float32
BF16 = mybir.dt.bfloat16
I32 = mybir.dt.int32
BF = ml_dtypes.bfloat16


def _register_prefix_mac():
    name = "PREFIX_MAC_GNN"
    if name in dve_ops._SUB_OPCODE_FOR_NAME:
        return next(op for op in dve_ops.OPS if op.name == name)
    spec = Spec(
        body=scan(AluOp.ADD, Src0 * Src1),
        reference=lambda in0, in1, s0, s1, imm2: np.cumsum(
            (in0.astype(np.float32) * in1).reshape(in0.shape[0], -1), axis=-1
        ),
    )
    shas = {}
    row = dve_ops._CUSTOM_DVE_ROW_BASE + len(dve_ops.OPS)
    for ver in ("v3", "v4"):
        uops = lower(spec, ver=ver)
        shas[ver] = DveOpSpec(name=name, opcode=row, uops=uops,
                              rd1_en=_has_src1(spec)).sha(ver)
    op = dve_ops.DveOp(name, spec, subdim=False, uops_sha=shas)
    dve_ops.OPS.append(op)
    dve_ops._SUB_OPCODE_FOR_NAME[name] = row
    dve_ops.CUSTOM_DVE_SPECS[name] = spec
    return op


def _host_prep(n_feat, e_feat, src, dst):
    """Sort edges by dst, shard by dst block, pad each (core, window) edge run
    onto a shared slot grid so the tile->window map is identical on all cores."""
    order = np.argsort(dst, kind="stable")
    src_s, dst_s, ef_s = src[order], dst[order], e_feat[order]

    # per (core, window) counts
    core_e = dst_s // NPC
    loc = dst_s - core_e * NPC
    win_e = loc // WIN
    cnt = np.zeros((N_CORES, N_WIN), dtype=np.int64)
    np.add.at(cnt, (core_e, win_e), 1)

    slot_cnt = cnt.max(axis=0)                       # shared grid
    G = np.concatenate([[0], np.cumsum(slot_cnt)])   # window slot boundaries
    total = int(G[-1])
    T = (total + 127) // 128                         # edge tiles per core
    E_PAD = T * 128

    # per-core padded edge arrays
    src_pad = np.zeros((N_CORES, E_PAD), dtype=np.int32)
    dstrel_pad = np.full((N_CORES, E_PAD), -1.0, dtype=np.float32)
    ef_pad = np.zeros((N_CORES, E_PAD, E_IN), dtype=np.float32)

    # tile -> window band
    w0 = np.zeros(T, dtype=np.int64)       # first window overlapping tile t
    bw = np.zeros(T, dtype=np.int64)       # how many windows overlap tile t
    for t in range(T):
        lo, hi = t * 128, min((t + 1) * 128, total)
        wlo = int(np.searchsorted(G, lo, side="right") - 1)
        whi = int(np.searchsorted(G, max(hi - 1, lo), side="right") - 1)
        wlo, whi = min(wlo, N_WIN - 1), min(whi, N_WIN - 1)
        w0[t] = wlo
        bw[t] = whi - wlo + 1
    B_W = int(bw.max())

    # fill padded arrays: window w of core k occupies slots [G[w], G[w]+cnt[k,w])
    core_starts = np.searchsorted(core_e, np.arange(N_CORES))
    for k in range(N_CORES):
        base = core_starts[k]
        cw = np.concatenate([[0], np.cumsum(cnt[k])])
        for w in range(N_WIN):
            s0, s1 = int(base + cw[w]), int(base + cw[w + 1])
            g0 = int(G[w])
            n_e = s1 - s0
            src_pad[k, g0:g0 + n_e] = src_s[s0:s1]
            ef_pad[k, g0:g0 + n_e] = ef_s[s0:s1]
            # dst_rel relative to the band anchor of the edge's tile
            slots = np.arange(g0, g0 + n_e)
            dstrel_pad[k, g0:g0 + n_e] = (
                loc[s0:s1] - w0[slots // 128] * WIN).astype(np.float32)

    # scatter pair list (t, w) from actual overlap, and per-window tile ranges
    pairs = []
    for t in range(T):
        for j in range(int(bw[t])):
            w = int(w0[t]) + j
            if w < N_WIN:
                pairs.append((t, w))
    win_tiles = {w: [t for (t, ww) in pairs if ww == w] for w in range(N_WIN)}

    grid = dict(T=T, E_PAD=E_PAD, B_W=B_W, w0=w0, bw=bw, win_tiles=win_tiles)

    per_core = []
    for k in range(N_CORES):
        per_core.append(dict(
            e_featT=np.ascontiguousarray(ef_pad[k].T),           # [10, E_PAD]
            n_featT=np.ascontiguousarray(n_feat[k * NPC:(k + 1) * NPC].T),  # [42, NPC]
            src_idx=np.ascontiguousarray(src_pad[k].reshape(T, 128).T).astype(np.int32),  # [128, T]
            dst_rel=np.ascontiguousarray(dstrel_pad[k].reshape(T, 128).T),  # [128, T]
        ))
    return grid, per_core


def _build_program(grid):
    T, B_W = grid["T"], grid["B_W"]
    w0, bw, win_tiles = grid["w0"], grid["bw"], grid["win_tiles"]
    PREFIX_MAC = _register_prefix_mac()

    nc = bacc.Bacc("TRN2", target_bir_lowering=False, debug=False,
                   num_devices=N_CORES)

    # ---- kernel I/O ----
    e_featT = nc.dram_tensor("e_featT", [E_IN, grid["E_PAD"]], F32, kind="ExternalInput")
    n_featT = nc.dram_tensor("n_featT", [D_IN, NPC], F32, kind="ExternalInput")
    src_idx = nc.dram_tensor("src_idx", [128, T], I32, kind="ExternalInput")
    dst_rel = nc.dram_tensor("dst_rel", [128, T], F32, kind="ExternalInput")
    iota = nc.dram_tensor("iota", [128, B_W * WIN], BF16, kind="ExternalInput")
    en1_w = nc.dram_tensor("en1_w", [E_IN, E_H], F32, kind="ExternalInput")
    en1_b = nc.dram_tensor("en1_b", [1, E_H], F32, kind="ExternalInput")
    en2_wp = nc.dram_tensor("en2_wp", [E_H, NW], BF16, kind="ExternalInput")
    en2_bp = nc.dram_tensor("en2_bp", [1, NW], BF16, kind="ExternalInput")
    lin0_wt = nc.dram_tensor("lin0_wt", [D_IN, D_H], F32, kind="ExternalInput")
    lin0_br = nc.dram_tensor("lin0_br", [1, D_H], F32, kind="ExternalInput")
    msgw_top = nc.dram_tensor("msgw_top", [D_H, D_H], F32, kind="ExternalInput")
    msgw_bot = nc.dram_tensor("msgw_bot", [D_H, D_H], F32, kind="ExternalInput")
    msgb_r = nc.dram_tensor("msgb_r", [1, D_H], F32, kind="ExternalInput")
    convb_r = nc.dram_tensor("convb_r", [1, D_H], F32, kind="ExternalInput")
    ident = nc.dram_tensor("ident", [D_H, D_H], F32, kind="ExternalInput")
    ones_r = nc.dram_tensor("ones_r", [1, 128], F32, kind="ExternalInput")
    ones_bf = nc.dram_tensor("ones_bf", [1, 128], BF16, kind="ExternalInput")
    y = nc.dram_tensor("y", [NPC, D_H], F32, kind="ExternalOutput")

    with tile.TileContext(nc) as tc:
        with (
            tc.tile_pool(name="const", bufs=1) as cpool,
            tc.tile_pool(name="dram", bufs=1, space="DRAM") as dram,
        ):
            # ---- persistent SBUF residents ----
            nfT_sb = cpool.tile([D_IN, NPC], F32)
            srci_sb = cpool.tile([128, T], I32)
            dstr_sb = cpool.tile([128, T], F32)
            iota_sb = cpool.tile([128, B_W * WIN], BF16)
            en1w_sb = cpool.tile([E_IN, E_H], F32)
            en1b_sb = cpool.tile([1, E_H], F32)
            en2wp_sb = cpool.tile([E_H, NW], BF16)
            en2bp_sb = cpool.tile([1, NW], BF16)
            lin0w_sb = cpool.tile([D_IN, D_H], F32)
            lin0b_sb = cpool.tile([1, D_H], F32)
            mwt_sb = cpool.tile([D_H, D_H], F32)
            mwb_sb = cpool.tile([D_H, D_H], F32)
            mb_sb = cpool.tile([1, D_H], F32)
            cvb_sb = cpool.tile([1, D_H], F32)
            id_sb = cpool.tile([D_H, D_H], F32)
            ones_sb = cpool.tile([1, 128], F32)
            onesbf_sb = cpool.tile([1, 128], BF16)
            outT_a = cpool.tile([D_H, NPC], F32)
            outT_b = cpool.tile([D_H, NPC], F32)
            pfx = cpool.tile([128, 1 + NW], F32)

            for sb, dr in [(nfT_sb, n_featT), (srci_sb, src_idx),
                           (dstr_sb, dst_rel), (iota_sb, iota), (en1w_sb, en1_w),
                           (en1b_sb, en1_b), (en2wp_sb, en2_wp), (en2bp_sb, en2_bp),
                           (lin0w_sb, lin0_wt), (lin0b_sb, lin0_br), (mwt_sb, msgw_top),
                           (mwb_sb, msgw_bot), (mb_sb, msgb_r), (cvb_sb, convb_r),
                           (id_sb, ident), (ones_sb, ones_r), (onesbf_sb, ones_bf)]:
                nc.sync.dma_start(sb[:], dr[:])
            nc.gpsimd.memset(pfx[:, 0:1], 0.0)

            # ---- DRAM scratch ----
            w_dram = dram.tile([T * 128, NW], BF16)
            cc_in = [dram.tile([NPC, D_H], BF16, name=f"cc_in{i}") for i in range(2)]
            cc_out = [dram.tile([N, D_H], BF16, name=f"cc_out{i}", addr_space="Shared")
                      for i in range(STEPS)]

            # =========== setup: build W' (bf16) in HBM ===========
            ECH = 16  # e_feat tiles per SBUF chunk
            with (
                tc.tile_pool(name="su_sb", bufs=3) as su_sb,
                tc.tile_pool(name="su_e", bufs=2) as su_e,
                tc.tile_pool(name="su_ph", bufs=2, space="PSUM") as su_ph,
                tc.tile_pool(name="su_pw", bufs=1, space="PSUM") as su_pw,
            ):
                pw = [su_pw.tile([128, 512], F32, name=f"pw{j}") for j in range(4)]
                nsz = [512, 512, 512, NW - 3 * 512]
                e_ch = None
                for t in range(T):
                    if t % ECH == 0:
                        c0 = t * 128
                        c1 = min((t + ECH) * 128, grid["E_PAD"])
                        e_ch = su_e.tile([E_IN, ECH * 128], F32, name="e_ch")
                        nc.sync.dma_start(e_ch[:, :c1 - c0], e_featT[:, c0:c1])
                    ph = su_ph.tile([128, 128], F32, name="ph")
                    o = (t % ECH) * 128
                    nc.tensor.matmul(ph[:], lhsT=en1w_sb[:], rhs=e_ch[:, o:o + 128],
                                     start=True, stop=False)
                    nc.tensor.matmul(ph[:], lhsT=en1b_sb[:], rhs=ones_sb[:1, :],
                                     start=False, stop=True)
                    h_sb = su_sb.tile([128, 128], BF16, name="h_sb")
                    nc.scalar.activation(h_sb[:], ph[:], mybir.ActivationFunctionType.Relu)
                    w_sb = su_sb.tile([128, NW], BF16, name="w_sb")
                    for j in range(4):
                        o0 = j * 512
                        # bias via K=1 matmul, then the bf16 edge-network matmul
                        nc.tensor.matmul(pw[j][:, :nsz[j]], lhsT=onesbf_sb[:1, :],
                                         rhs=en2bp_sb[:, o0:o0 + nsz[j]],
                                         start=True, stop=False)
                        nc.tensor.matmul(pw[j][:, :nsz[j]], lhsT=h_sb[:],
                                         rhs=en2wp_sb[:, o0:o0 + nsz[j]],
                                         start=False, stop=True)
                        # cast fp32 PSUM -> bf16 SBUF, split across DVE/Scalar
                        if j < 2:
                            nc.vector.tensor_scalar_add(
                                out=w_sb[:, o0:o0 + nsz[j]],
                                in0=pw[j][:, :nsz[j]], scalar1=0.0)
                        else:
                            nc.scalar.copy(w_sb[:, o0:o0 + nsz[j]],
                                           pw[j][:, :nsz[j]])
                    nc.sync.dma_start(w_dram[t * 128:(t + 1) * 128, :], w_sb[:])

            # =========== step pools ===========
            with (
                tc.tile_pool(name="st_w", bufs=16) as p_w,
                tc.tile_pool(name="st_x", bufs=8) as p_x,
                tc.tile_pool(name="st_m", bufs=4) as p_m,
                tc.tile_pool(name="st_oh", bufs=4) as p_oh,
                tc.tile_pool(name="st_sm", bufs=4) as p_sm,
                tc.tile_pool(name="ps_ag", bufs=3, space="PSUM") as ps_ag,
                tc.tile_pool(name="ps_up", bufs=2, space="PSUM") as ps_up,
                tc.tile_pool(name="ps_tr", bufs=1, space="PSUM") as ps_tr,
            ):
                def window_cols(w):
                    n0 = w * WIN
                    m = min(WIN, NPC - n0)
                    return n0, m

                def update_window(w, outT_cur, outT_new, aggr_ps, step):
                    """Window epilogue: finish aggr, relu, update matmul, transpose, DMA."""
                    n0, m = window_cols(w)
                    last = step == STEPS
                    # + out (identity residual into conv) and conv bias
                    nc.tensor.matmul(aggr_ps[:, :m], lhsT=id_sb[:],
                                     rhs=outT_cur[:, n0:n0 + m], start=False, stop=False)
                    nc.tensor.matmul(aggr_ps[:, :m], lhsT=cvb_sb[:],
                                     rhs=ones_sb[:1, :m], start=False, stop=True)
                    mT_sb = p_sm.tile([D_H, WIN], F32, name="mT_sb")
                    nc.scalar.activation(mT_sb[:, :m], aggr_ps[:, :m],
                                         mybir.ActivationFunctionType.Relu)
                    up = ps_up.tile([D_H, WIN], F32, name="up")
                    nc.tensor.matmul(up[:, :m], lhsT=mwt_sb[:], rhs=mT_sb[:, :m],
                                     start=True, stop=False)
                    nc.tensor.matmul(up[:, :m], lhsT=mwb_sb[:], rhs=outT_cur[:, n0:n0 + m],
                                     start=False, stop=False)
                    nc.tensor.matmul(up[:, :m], lhsT=mb_sb[:], rhs=ones_sb[:1, :m],
                                     start=False, stop=not last)
                    if last:
                        nc.tensor.matmul(up[:, :m], lhsT=id_sb[:], rhs=nfT_sb[:, n0:n0 + m],
                                         start=False, stop=True)
                    nc.scalar.copy(outT_new[:, n0:n0 + m], up[:, :m])
                    tr = ps_tr.tile([128, D_H], F32, name="tr")
                    nc.tensor.transpose(tr[:m, :], outT_new[:, n0:n0 + m], id_sb[:])
                    if last:
                        rows = p_sm.tile([128, D_H], F32, name="rows_f")
                        nc.scalar.copy(rows[:m, :], tr[:m, :])
                        nc.sync.dma_start(y[n0:n0 + m, :], rows[:m, :])
                    else:
                        rows = p_sm.tile([128, D_H], BF16, name="rows_b")
                        nc.scalar.copy(rows[:m, :], tr[:m, :])
                        nc.sync.dma_start(cc_in[step % 2][n0:n0 + m, :], rows[:m, :])

                def all_gather(step):
                    nc.gpsimd.collective_compute(
                        "AllGather", mybir.AluOpType.bypass,
                        replica_groups=[list(range(N_CORES))],
                        ins=[cc_in[step % 2].opt()], outs=[cc_out[step].opt()])

                # =========== lin0: out0 = relu(n_feat @ lin0_w + b) ===========
                for w in range(N_WIN):
                    n0, m = window_cols(w)
                    up = ps_up.tile([D_H, WIN], F32, name="up")
                    nc.tensor.matmul(up[:, :m], lhsT=lin0w_sb[:], rhs=nfT_sb[:, n0:n0 + m],
                                     start=True, stop=False)
                    nc.tensor.matmul(up[:, :m], lhsT=lin0b_sb[:], rhs=ones_sb[:1, :m],
                                     start=False, stop=True)
                    nc.scalar.activation(outT_a[:, n0:n0 + m], up[:, :m],
                                         mybir.ActivationFunctionType.Relu)
                    tr = ps_tr.tile([128, D_H], F32, name="tr")
                    nc.tensor.transpose(tr[:m, :], outT_a[:, n0:n0 + m], id_sb[:])
                    rows = p_sm.tile([128, D_H], BF16, name="rows_b")
                    nc.scalar.copy(rows[:m, :], tr[:m, :])
                    nc.sync.dma_start(cc_in[0][n0:n0 + m, :], rows[:m, :])
                all_gather(0)

                # =========== message passing steps ===========
                for step in range(1, STEPS + 1):
                    outT_cur = outT_a if step % 2 == 1 else outT_b
                    outT_new = outT_b if step % 2 == 1 else outT_a
                    src_buf = cc_out[step - 1]
                    aggr_of = {}
                    for t in range(T):
                        x_g = p_x.tile([128, D_H], BF16, name="x_g")
                        nc.gpsimd.indirect_dma_start(
                            out=x_g[:], out_offset=None, in_=src_buf[:],
                            in_offset=bass.IndirectOffsetOnAxis(
                                ap=srci_sb[:, t:t + 1], axis=0))
                        w_t = p_w.tile([128, NW], BF16, name="w_t")
                        nc.sync.dma_start(w_t[:], w_dram[t * 128:(t + 1) * 128, :])
                        nc.vector._custom_dve(
                            PREFIX_MAC, out=pfx[:, 1:1 + NW], in0=w_t[:],
                            in1=x_g[:, None, :].to_broadcast([128, D_H, D_H]))
                        msg = p_m.tile([128, D_H], BF16, name="msg")
                        nc.vector.tensor_tensor(
                            out=msg[:], in0=pfx[:, D_H:1 + NW:D_H],
                            in1=pfx[:, 0:NW:D_H], op=mybir.AluOpType.subtract)
                        bwt = int(bw[t])
                        oh = p_oh.tile([128, B_W * WIN], BF16, name="oh")
                        nc.vector.tensor_scalar(
                            out=oh[:, :bwt * WIN], in0=iota_sb[:, :bwt * WIN],
                            scalar1=dstr_sb[:, t:t + 1],
                            scalar2=None, op0=mybir.AluOpType.is_equal)
                        # scatter matmuls
                        for j in range(bwt):
                            w = int(w0[t]) + j
                            if w >= N_WIN:
                                continue
                            tiles_w = win_tiles[w]
                            if w not in aggr_of:
                                aggr_of[w] = ps_ag.tile([D_H, WIN], F32, name="aggr")
                            first = t == tiles_w[0]
                            last_t = t == tiles_w[-1]
                            nc.tensor.matmul(aggr_of[w][:], lhsT=msg[:],
                                             rhs=oh[:, j * WIN:(j + 1) * WIN],
                                             start=first, stop=False)
                            if last_t:
                                update_window(w, outT_cur, outT_new,
                                              aggr_of.pop(w), step)
                    if step < STEPS:
                        all_gather(step)

    nc.compile()
    return nc


_CACHED = {}


def kernel(n_feat, e_feat, src, dst, lin0_w, lin0_b, en1_w, en1_b,
           en2_w, en2_b, conv_bias, msg_w, msg_b):
    n_feat = np.asarray(n_feat, dtype=np.float32)
    e_feat = np.asarray(e_feat, dtype=np.float32)
    src = np.asarray(src, dtype=np.int32)
    dst = np.asarray(dst, dtype=np.int32)

    grid, per_core = _host_prep(n_feat, e_feat, src, dst)

    key = (grid["T"], grid["B_W"], tuple(grid["w0"].tolist()))
    if key not in _CACHED:
        _CACHED.clear()
        _CACHED[key] = _build_program(grid)
    nc = _CACHED[key]

    en2_wp = np.ascontiguousarray(
        np.asarray(en2_w, np.float32).reshape(E_H, D_H, D_H).transpose(0, 2, 1).reshape(E_H, NW))
    shared = dict(
        iota=np.tile(np.arange(grid["B_W"] * WIN, dtype=np.float32), (128, 1)).astype(BF),
        en1_w=np.asarray(en1_w, np.float32),
        en1_b=np.asarray(en1_b, np.float32).reshape(1, E_H),
        en2_wp=en2_wp.astype(BF),
        en2_bp=np.ascontiguousarray(
            np.asarray(en2_b, np.float32).reshape(D_H, D_H).T.reshape(1, NW)).astype(BF),
        lin0_wt=np.asarray(lin0_w, np.float32),
        lin0_br=np.asarray(lin0_b, np.float32).reshape(1, D_H),
        msgw_top=np.ascontiguousarray(np.asarray(msg_w, np.float32)[:D_H, :]),
        msgw_bot=np.ascontiguousarray(np.asarray(msg_w, np.float32)[D_H:, :]),
        msgb_r=np.asarray(msg_b, np.float32).reshape(1, D_H),
        convb_r=np.asarray(conv_bias, np.float32).reshape(1, D_H),
        ident=np.eye(D_H, dtype=np.float32),
        ones_r=np.ones((1, 128), dtype=np.float32),
        ones_bf=np.ones((1, 128), dtype=BF),
    )
    in_maps = []
    for k in range(N_CORES):
        m = dict(shared)
        m.update(per_core[k])
        in_maps.append(m)

    res = bass_utils.run_bass_kernel_spmd(nc, in_maps, core_ids=list(range(N_CORES)))
    out = np.concatenate([res.results[k]["y"] for k in range(N_CORES)], axis=0)
    return out.astype(np.float32)


# revision 23
# speedup vs baseline: 1.4083x; 1.0525x over previous
"""Trainium2 Bass kernel for nn_GatherModel (NNConv GNN message passing).

8-core SPMD, edge-parallel sharded by destination node block:
  - core k owns nodes [k*6250, (k+1)*6250) and all edges whose dst lies there
  - per-edge weight matrices W'_e (o-major) are built once on device (PE,
    bf16) and streamed bf16 from HBM each of the 6 message-passing steps
  - per-edge contraction msg = x_src @ W_e runs on the Vector engine via a
    custom fused multiply+prefix-scan DVE op (bf16 in, fp32 out),
    extracting per-o sums by strided prefix differences
  - scatter (segment-sum over dst) is a PE matmul against on-device-built
    bf16 one-hot window matrices; node update runs fp32 in transposed
    feature layout
  - each step ends with an 8-core AllGather of bf16 node features
"""
import numpy as np
import ml_dtypes

import concourse.bacc as bacc
import concourse.bass as bass
import concourse.mybir as mybir
import concourse.tile as tile
from concourse import bass_utils, dve_ops
from concourse.dve_spec import Spec, Src0, Src1, scan, AluOp, lower, _has_src1
from concourse.dve_uop import DveOpSpec

N = 50000
E = 150000
D_IN = 42
D_H = 42
E_IN = 10
E_H = 128
STEPS = 6
N_CORES = 8
NPC = N // N_CORES          # 6250 nodes per core
WIN = 128                   # scatter window (node block) size
N_WIN = (NPC + WIN - 1) // WIN  # 49 windows per core, last partial (106)
NW = D_H * D_H              # 1764
F32 = mybir.dt.float32
BF16 = mybir.dt.bfloat16
I32 = mybir.dt.int32
BF = ml_dtypes.bfloat16


def _register_prefix_mac():
    name = "PREFIX_MAC_GNN"
    if name in dve_ops._SUB_OPCODE_FOR_NAME:
        return next(op for op in dve_ops.OPS if op.name == name)
    spec = Spec(
        body=scan(AluOp.ADD, Src0 * Src1),
        reference=lambda in0, in1, s0, s1, imm2: np.cumsum(
            (in0.astype(np.float32) * in1).reshape(in0.shape[0], -1), axis=-1
        ),
    )
    shas = {}
    row = dve_ops._CUSTOM_DVE_ROW_BASE + len(dve_ops.OPS)
    for ver in ("v3", "v4"):
        uops = lower(spec, ver=ver)
        shas[ver] = DveOpSpec(name=name, opcode=row, uops=uops,
                              rd1_en=_has_src1(spec)).sha(ver)
    op = dve_ops.DveOp(name, spec, subdim=False, uops_sha=shas)
    dve_ops.OPS.append(op)
    dve_ops._SUB_OPCODE_FOR_NAME[name] = row
    dve_ops.CUSTOM_DVE_SPECS[name] = spec
    return op


def _host_prep(n_feat, e_feat, src, dst):
    """Sort edges by dst, shard by dst block, pad each (core, window) edge run
    onto a shared slot grid so the tile->window map is identical on all cores."""
    order = np.argsort(dst, kind="stable")
    src_s, dst_s, ef_s = src[order], dst[order], e_feat[order]

    # per (core, window) counts
    core_e = dst_s // NPC
    loc = dst_s - core_e * NPC
    win_e = loc // WIN
    cnt = np.zeros((N_CORES, N_WIN), dtype=np.int64)
    np.add.at(cnt, (core_e, win_e), 1)

    slot_cnt = cnt.max(axis=0)                       # shared grid
    G = np.concatenate([[0], np.cumsum(slot_cnt)])   # window slot boundaries
    total = int(G[-1])
    T = (total + 127) // 128                         # edge tiles per core
    E_PAD = T * 128

    # per-core padded edge arrays
    src_pad = np.zeros((N_CORES, E_PAD), dtype=np.int32)
    dstrel_pad = np.full((N_CORES, E_PAD), -1.0, dtype=np.float32)
    ef_pad = np.zeros((N_CORES, E_PAD, E_IN), dtype=np.float32)

    # tile -> window band
    w0 = np.zeros(T, dtype=np.int64)       # first window overlapping tile t
    bw = np.zeros(T, dtype=np.int64)       # how many windows overlap tile t
    for t in range(T):
        lo, hi = t * 128, min((t + 1) * 128, total)
        wlo = int(np.searchsorted(G, lo, side="right") - 1)
        whi = int(np.searchsorted(G, max(hi - 1, lo), side="right") - 1)
        wlo, whi = min(wlo, N_WIN - 1), min(whi, N_WIN - 1)
        w0[t] = wlo
        bw[t] = whi - wlo + 1
    B_W = int(bw.max())

    # fill padded arrays: window w of core k occupies slots [G[w], G[w]+cnt[k,w])
    core_starts = np.searchsorted(core_e, np.arange(N_CORES))
    for k in range(N_CORES):
        base = core_starts[k]
        cw = np.concatenate([[0], np.cumsum(cnt[k])])
        for w in range(N_WIN):
            s0, s1 = int(base + cw[w]), int(base + cw[w + 1])
            g0 = int(G[w])
            n_e = s1 - s0
            src_pad[k, g0:g0 + n_e] = src_s[s0:s1]
            ef_pad[k, g0:g0 + n_e] = ef_s[s0:s1]
            # dst_rel relative to the band anchor of the edge's tile
            slots = np.arange(g0, g0 + n_e)
            dstrel_pad[k, g0:g0 + n_e] = (
                loc[s0:s1] - w0[slots // 128] * WIN).astype(np.float32)

    # scatter pair list (t, w) from actual overlap, and per-window tile ranges
    pairs = []
    for t in range(T):
        for j in range(int(bw[t])):
            w = int(w0[t]) + j
            if w < N_WIN:
                pairs.append((t, w))
    win_tiles = {w: [t for (t, ww) in pairs if ww == w] for w in range(N_WIN)}

    grid = dict(T=T, E_PAD=E_PAD, B_W=B_W, w0=w0, bw=bw, win_tiles=win_tiles)

    per_core = []
    for k in range(N_CORES):
        per_core.append(dict(
            e_featT=np.ascontiguousarray(ef_pad[k].T),           # [10, E_PAD]
            n_featT=np.ascontiguousarray(n_feat[k * NPC:(k + 1) * NPC].T),  # [42, NPC]
            src_idx=np.ascontiguousarray(src_pad[k].reshape(T, 128).T).astype(np.int32),  # [128, T]
            dst_rel=np.ascontiguousarray(dstrel_pad[k].reshape(T, 128).T),  # [128, T]
        ))
    return grid, per_core


def _build_program(grid):
    T, B_W = grid["T"], grid["B_W"]
    w0, bw, win_tiles = grid["w0"], grid["bw"], grid["win_tiles"]
    PREFIX_MAC = _register_prefix_mac()

    # per-tile column offsets into the resident one-hot bank
    oh_off = np.zeros(T + 1, dtype=np.int64)
    for t in range(T):
        oh_off[t + 1] = oh_off[t] + int(bw[t]) * WIN
    OH_COLS = int(oh_off[T])

    nc = bacc.Bacc("TRN2", target_bir_lowering=False, debug=False,
                   num_devices=N_CORES)

    # ---- kernel I/O ----
    e_featT = nc.dram_tensor("e_featT", [E_IN, grid["E_PAD"]], F32, kind="ExternalInput")
    n_featT = nc.dram_tensor("n_featT", [D_IN, NPC], F32, kind="ExternalInput")
    src_idx = nc.dram_tensor("src_idx", [128, T], I32, kind="ExternalInput")
    dst_rel = nc.dram_tensor("dst_rel", [128, T], F32, kind="ExternalInput")
    iota = nc.dram_tensor("iota", [128, B_W * WIN], BF16, kind="ExternalInput")
    en1_w = nc.dram_tensor("en1_w", [E_IN, E_H], F32, kind="ExternalInput")
    en1_b = nc.dram_tensor("en1_b", [1, E_H], F32, kind="ExternalInput")
    en2_wp = nc.dram_tensor("en2_wp", [E_H, NW], BF16, kind="ExternalInput")
    en2_bp = nc.dram_tensor("en2_bp", [1, NW], BF16, kind="ExternalInput")
    lin0_wt = nc.dram_tensor("lin0_wt", [D_IN, D_H], F32, kind="ExternalInput")
    lin0_br = nc.dram_tensor("lin0_br", [1, D_H], F32, kind="ExternalInput")
    msgw_top = nc.dram_tensor("msgw_top", [D_H, D_H], F32, kind="ExternalInput")
    msgw_bot = nc.dram_tensor("msgw_bot", [D_H, D_H], F32, kind="ExternalInput")
    msgb_r = nc.dram_tensor("msgb_r", [1, D_H], F32, kind="ExternalInput")
    convb_r = nc.dram_tensor("convb_r", [1, D_H], F32, kind="ExternalInput")
    ident = nc.dram_tensor("ident", [D_H, D_H], F32, kind="ExternalInput")
    ones_r = nc.dram_tensor("ones_r", [1, 128], F32, kind="ExternalInput")
    ones_bf = nc.dram_tensor("ones_bf", [1, 128], BF16, kind="ExternalInput")
    y = nc.dram_tensor("y", [NPC, D_H], F32, kind="ExternalOutput")

    with tile.TileContext(nc) as tc:
        with (
            tc.tile_pool(name="const", bufs=1) as cpool,
            tc.tile_pool(name="dram", bufs=1, space="DRAM") as dram,
        ):
            # ---- persistent SBUF residents ----
            nfT_sb = cpool.tile([D_IN, NPC], F32)
            srci_sb = cpool.tile([128, T], I32)
            dstr_sb = cpool.tile([128, T], F32)
            iota_sb = cpool.tile([128, B_W * WIN], BF16)
            en1w_sb = cpool.tile([E_IN, E_H], F32)
            en1b_sb = cpool.tile([1, E_H], F32)
            en2wp_sb = cpool.tile([E_H, NW], BF16)
            en2bp_sb = cpool.tile([1, NW], BF16)
            lin0w_sb = cpool.tile([D_IN, D_H], F32)
            lin0b_sb = cpool.tile([1, D_H], F32)
            mwt_sb = cpool.tile([D_H, D_H], F32)
            mwb_sb = cpool.tile([D_H, D_H], F32)
            mb_sb = cpool.tile([1, D_H], F32)
            cvb_sb = cpool.tile([1, D_H], F32)
            id_sb = cpool.tile([D_H, D_H], F32)
            ones_sb = cpool.tile([1, 128], F32)
            onesbf_sb = cpool.tile([1, 128], BF16)
            outT_a = cpool.tile([D_H, NPC], F32)
            outT_b = cpool.tile([D_H, NPC], F32)
            pfx = cpool.tile([128, 1 + NW], F32)

            for sb, dr in [(nfT_sb, n_featT), (srci_sb, src_idx),
                           (dstr_sb, dst_rel), (iota_sb, iota), (en1w_sb, en1_w),
                           (en1b_sb, en1_b), (en2wp_sb, en2_wp), (en2bp_sb, en2_bp),
                           (lin0w_sb, lin0_wt), (lin0b_sb, lin0_br), (mwt_sb, msgw_top),
                           (mwb_sb, msgw_bot), (mb_sb, msgb_r), (cvb_sb, convb_r),
                           (id_sb, ident), (ones_sb, ones_r), (onesbf_sb, ones_bf)]:
                nc.sync.dma_start(sb[:], dr[:])
            nc.gpsimd.memset(pfx[:, 0:1], 0.0)

            # ---- DRAM scratch ----
            w_dram = dram.tile([T * 128, NW], BF16)
            oh_dram = dram.tile([128, OH_COLS], BF16)
            cc_in = [dram.tile([NPC, D_H], BF16, name=f"cc_in{i}") for i in range(2)]
            cc_out = [dram.tile([N, D_H], BF16, name=f"cc_out{i}", addr_space="Shared")
                      for i in range(STEPS)]

            # =========== setup: build W' (bf16) in HBM ===========
            ECH = 16  # e_feat tiles per SBUF chunk
            with (
                tc.tile_pool(name="su_h", bufs=1) as su_h,
                tc.tile_pool(name="su_sb", bufs=3) as su_sb,
                tc.tile_pool(name="su_e", bufs=2) as su_e,
                tc.tile_pool(name="su_ph", bufs=2, space="PSUM") as su_ph,
                tc.tile_pool(name="su_pw", bufs=3, space="PSUM") as su_pw,
            ):
                # phase 0: h_all = relu(e_feat @ en1_w + b) for every edge tile,
                # and the step-invariant one-hot scatter bank on the idle DVE
                h_all = su_h.tile([128, T * 128], BF16)
                e_ch = None
                for t in range(T):
                    if t % ECH == 0:
                        c0 = t * 128
                        c1 = min((t + ECH) * 128, grid["E_PAD"])
                        e_ch = su_e.tile([E_IN, ECH * 128], F32, name="e_ch")
                        nc.sync.dma_start(e_ch[:, :c1 - c0], e_featT[:, c0:c1])
                    ph = su_ph.tile([128, 128], F32, name="ph")
                    o = (t % ECH) * 128
                    nc.tensor.matmul(ph[:], lhsT=en1w_sb[:], rhs=e_ch[:, o:o + 128],
                                     start=True, stop=False)
                    nc.tensor.matmul(ph[:], lhsT=en1b_sb[:], rhs=ones_sb[:1, :],
                                     start=False, stop=True)
                    nc.scalar.activation(h_all[:, t * 128:(t + 1) * 128], ph[:],
                                         mybir.ActivationFunctionType.Relu)
                    bwt = int(bw[t])
                    oh_sb = su_sb.tile([128, B_W * WIN], BF16, name="oh_sb")
                    nc.vector.tensor_scalar(
                        out=oh_sb[:, :bwt * WIN], in0=iota_sb[:, :bwt * WIN],
                        scalar1=dstr_sb[:, t:t + 1],
                        scalar2=None, op0=mybir.AluOpType.is_equal)
                    nc.sync.dma_start(
                        oh_dram[:, int(oh_off[t]):int(oh_off[t]) + bwt * WIN],
                        oh_sb[:, :bwt * WIN])

                # bias broadcast (chunks 0-1 only; chunks 2-3 get bias via PE)
                bias_sb = su_sb.tile([128, 1024], BF16, name="bias_sb")
                for j in range(2):
                    o0 = j * 512
                    pb = su_pw.tile([128, 512], F32, name="pb")
                    nc.tensor.matmul(pb[:], lhsT=onesbf_sb[:1, :],
                                     rhs=en2bp_sb[:, o0:o0 + 512],
                                     start=True, stop=True)
                    nc.vector.tensor_scalar_add(out=bias_sb[:, o0:o0 + 512],
                                                in0=pb[:], scalar1=0.0)

                nsz = [512, 512, 512, NW - 3 * 512]
                for t in range(T):
                    h_t = h_all[:, t * 128:(t + 1) * 128]
                    w_sb = su_sb.tile([128, NW], BF16, name="w_sb")
                    for j in range(4):
                        o0 = j * 512
                        pw = su_pw.tile([128, 512], F32, name="pw")
                        if j < 2:
                            # bias added on DVE together with the bf16 cast
                            nc.tensor.matmul(pw[:, :nsz[j]], lhsT=h_t,
                                             rhs=en2wp_sb[:, o0:o0 + nsz[j]],
                                             start=True, stop=True)
                            nc.vector.tensor_tensor(
                                out=w_sb[:, o0:o0 + nsz[j]], in0=pw[:, :nsz[j]],
                                in1=bias_sb[:, o0:o0 + nsz[j]],
                                op=mybir.AluOpType.add)
                        else:
                            # bias via K=1 matmul; cast on Scalar
                            nc.tensor.matmul(pw[:, :nsz[j]], lhsT=onesbf_sb[:1, :],
                                             rhs=en2bp_sb[:, o0:o0 + nsz[j]],
                                             start=True, stop=False)
                            nc.tensor.matmul(pw[:, :nsz[j]], lhsT=h_t,
                                             rhs=en2wp_sb[:, o0:o0 + nsz[j]],
                                             start=False, stop=True)
                            nc.scalar.copy(w_sb[:, o0:o0 + nsz[j]],
                                           pw[:, :nsz[j]])
                    nc.sync.dma_start(w_dram[t * 128:(t + 1) * 128, :], w_sb[:])

            # =========== step pools ===========
            with (
                tc.tile_pool(name="st_w", bufs=16) as p_w,
                tc.tile_pool(name="st_x", bufs=8) as p_x,
                tc.tile_pool(name="st_m", bufs=4) as p_m,
                tc.tile_pool(name="st_oh", bufs=6) as p_oh,
                tc.tile_pool(name="st_sm", bufs=4) as p_sm,
                tc.tile_pool(name="ps_ag", bufs=3, space="PSUM") as ps_ag,
                tc.tile_pool(name="ps_up", bufs=2, space="PSUM") as ps_up,
                tc.tile_pool(name="ps_tr", bufs=1, space="PSUM") as ps_tr,
            ):
                def window_cols(w):
                    n0 = w * WIN
                    m = min(WIN, NPC - n0)
                    return n0, m

                def update_window(w, outT_cur, outT_new, aggr_ps, step):
                    """Window epilogue: finish aggr, relu, update matmul, transpose, DMA."""
                    n0, m = window_cols(w)
                    last = step == STEPS
                    # + out (identity residual into conv) and conv bias
                    nc.tensor.matmul(aggr_ps[:, :m], lhsT=id_sb[:],
                                     rhs=outT_cur[:, n0:n0 + m], start=False, stop=False)
                    nc.tensor.matmul(aggr_ps[:, :m], lhsT=cvb_sb[:],
                                     rhs=ones_sb[:1, :m], start=False, stop=True)
                    mT_sb = p_sm.tile([D_H, WIN], F32, name="mT_sb")
                    nc.scalar.activation(mT_sb[:, :m], aggr_ps[:, :m],
                                         mybir.ActivationFunctionType.Relu)
                    up = ps_up.tile([D_H, WIN], F32, name="up")
                    nc.tensor.matmul(up[:, :m], lhsT=mwt_sb[:], rhs=mT_sb[:, :m],
                                     start=True, stop=False)
                    nc.tensor.matmul(up[:, :m], lhsT=mwb_sb[:], rhs=outT_cur[:, n0:n0 + m],
                                     start=False, stop=False)
                    nc.tensor.matmul(up[:, :m], lhsT=mb_sb[:], rhs=ones_sb[:1, :m],
                                     start=False, stop=not last)
                    if last:
                        nc.tensor.matmul(up[:, :m], lhsT=id_sb[:], rhs=nfT_sb[:, n0:n0 + m],
                                         start=False, stop=True)
                    nc.scalar.copy(outT_new[:, n0:n0 + m], up[:, :m])
                    tr = ps_tr.tile([128, D_H], F32, name="tr")
                    nc.tensor.transpose(tr[:m, :], outT_new[:, n0:n0 + m], id_sb[:])
                    if last:
                        rows = p_sm.tile([128, D_H], F32, name="rows_f")
                        nc.scalar.copy(rows[:m, :], tr[:m, :])
                        nc.sync.dma_start(y[n0:n0 + m, :], rows[:m, :])
                    else:
                        rows = p_sm.tile([128, D_H], BF16, name="rows_b")
                        nc.scalar.copy(rows[:m, :], tr[:m, :])
                        nc.sync.dma_start(cc_in[step % 2][n0:n0 + m, :],
                                          rows[:m, :])

                def all_gather(step):
                    nc.gpsimd.collective_compute(
                        "AllGather", mybir.AluOpType.bypass,
                        replica_groups=[list(range(N_CORES))],
                        ins=[cc_in[step % 2].opt()], outs=[cc_out[step].opt()])

                # =========== lin0: out0 = relu(n_feat @ lin0_w + b) ===========
                for w in range(N_WIN):
                    n0, m = window_cols(w)
                    up = ps_up.tile([D_H, WIN], F32, name="up")
                    nc.tensor.matmul(up[:, :m], lhsT=lin0w_sb[:], rhs=nfT_sb[:, n0:n0 + m],
                                     start=True, stop=False)
                    nc.tensor.matmul(up[:, :m], lhsT=lin0b_sb[:], rhs=ones_sb[:1, :m],
                                     start=False, stop=True)
                    nc.scalar.activation(outT_a[:, n0:n0 + m], up[:, :m],
                                         mybir.ActivationFunctionType.Relu)
                    tr = ps_tr.tile([128, D_H], F32, name="tr")
                    nc.tensor.transpose(tr[:m, :], outT_a[:, n0:n0 + m], id_sb[:])
                    rows = p_sm.tile([128, D_H], BF16, name="rows_b")
                    nc.scalar.copy(rows[:m, :], tr[:m, :])
                    nc.sync.dma_start(cc_in[0][n0:n0 + m, :], rows[:m, :])
                all_gather(0)

                # =========== message passing steps ===========
                for step in range(1, STEPS + 1):
                    outT_cur = outT_a if step % 2 == 1 else outT_b
                    outT_new = outT_b if step % 2 == 1 else outT_a
                    src_buf = cc_out[step - 1]
                    aggr_of = {}
                    for t in range(T):
                        x_g = p_x.tile([128, D_H], BF16, name="x_g")
                        nc.gpsimd.indirect_dma_start(
                            out=x_g[:], out_offset=None, in_=src_buf[:],
                            in_offset=bass.IndirectOffsetOnAxis(
                                ap=srci_sb[:, t:t + 1], axis=0))
                        w_t = p_w.tile([128, NW], BF16, name="w_t")
                        nc.sync.dma_start(w_t[:], w_dram[t * 128:(t + 1) * 128, :])
                        nc.vector._custom_dve(
                            PREFIX_MAC, out=pfx[:, 1:1 + NW], in0=w_t[:],
                            in1=x_g[:, None, :].to_broadcast([128, D_H, D_H]))
                        msg = p_m.tile([128, D_H], BF16, name="msg")
                        nc.vector.tensor_tensor(
                            out=msg[:], in0=pfx[:, D_H:1 + NW:D_H],
                            in1=pfx[:, 0:NW:D_H], op=mybir.AluOpType.subtract)
                        bwt = int(bw[t])
                        # scatter matmuls against the streamed one-hot bank
                        oh = p_oh.tile([128, B_W * WIN], BF16, name="oh")
                        nc.sync.dma_start(
                            oh[:, :bwt * WIN],
                            oh_dram[:, int(oh_off[t]):int(oh_off[t]) + bwt * WIN])
                        for j in range(bwt):
                            w = int(w0[t]) + j
                            if w >= N_WIN:
                                continue
                            tiles_w = win_tiles[w]
                            if w not in aggr_of:
                                aggr_of[w] = ps_ag.tile([D_H, WIN], F32, name="aggr")
                            first = t == tiles_w[0]
                            last_t = t == tiles_w[-1]
                            nc.tensor.matmul(aggr_of[w][:], lhsT=msg[:],
                                             rhs=oh[:, j * WIN:(j + 1) * WIN],
                                             start=first, stop=False)
                            if last_t:
                                update_window(w, outT_cur, outT_new,
                                              aggr_of.pop(w), step)
                    if step < STEPS:
                        all_gather(step)

    nc.compile()
    return nc


_CACHED = {}


def kernel(n_feat, e_feat, src, dst, lin0_w, lin0_b, en1_w, en1_b,
           en2_w, en2_b, conv_bias, msg_w, msg_b):
    n_feat = np.asarray(n_feat, dtype=np.float32)
    e_feat = np.asarray(e_feat, dtype=np.float32)
    src = np.asarray(src, dtype=np.int32)
    dst = np.asarray(dst, dtype=np.int32)

    grid, per_core = _host_prep(n_feat, e_feat, src, dst)

    key = (grid["T"], grid["B_W"], tuple(grid["w0"].tolist()))
    if key not in _CACHED:
        _CACHED.clear()
        _CACHED[key] = _build_program(grid)
    nc = _CACHED[key]

    en2_wp = np.ascontiguousarray(
        np.asarray(en2_w, np.float32).reshape(E_H, D_H, D_H).transpose(0, 2, 1).reshape(E_H, NW))
    shared = dict(
        iota=np.tile(np.arange(grid["B_W"] * WIN, dtype=np.float32), (128, 1)).astype(BF),
        en1_w=np.asarray(en1_w, np.float32),
        en1_b=np.asarray(en1_b, np.float32).reshape(1, E_H),
        en2_wp=en2_wp.astype(BF),
        en2_bp=np.ascontiguousarray(
            np.asarray(en2_b, np.float32).reshape(D_H, D_H).T.reshape(1, NW)).astype(BF),
        lin0_wt=np.asarray(lin0_w, np.float32),
        lin0_br=np.asarray(lin0_b, np.float32).reshape(1, D_H),
        msgw_top=np.ascontiguousarray(np.asarray(msg_w, np.float32)[:D_H, :]),
        msgw_bot=np.ascontiguousarray(np.asarray(msg_w, np.float32)[D_H:, :]),
        msgb_r=np.asarray(msg_b, np.float32).reshape(1, D_H),
        convb_r=np.asarray(conv_bias, np.float32).reshape(1, D_H),
        ident=np.eye(D_H, dtype=np.float32),
        ones_r=np.ones((1, 128), dtype=np.float32),
        ones_bf=np.ones((1, 128), dtype=BF),
    )
    in_maps = []
    for k in range(N_CORES):
        m = dict(shared)
        m.update(per_core[k])
        in_maps.append(m)

    res = bass_utils.run_bass_kernel_spmd(nc, in_maps, core_ids=list(range(N_CORES)))
    out = np.concatenate([res.results[k]["y"] for k in range(N_CORES)], axis=0)
    return out.astype(np.float32)


# revision 33
# speedup vs baseline: 1.4637x; 1.0393x over previous
"""Trainium2 Bass kernel for nn_GatherModel (NNConv GNN message passing).

8-core SPMD, edge-parallel sharded by destination node block:
  - core k owns nodes [k*6250, (k+1)*6250) and all edges whose dst lies there
  - per-edge weight matrices W'_e (o-major) are built once on device (PE,
    bf16) and streamed bf16 from HBM each of the 6 message-passing steps
  - per-edge contraction msg = x_src @ W_e runs on the Vector engine via a
    custom fused multiply+prefix-scan DVE op (bf16 in, fp32 out),
    extracting per-o sums by strided prefix differences
  - scatter (segment-sum over dst) is a PE matmul against on-device-built
    bf16 one-hot window matrices; node update runs fp32 in transposed
    feature layout
  - each step ends with an 8-core AllGather of bf16 node features
"""
import numpy as np
import ml_dtypes

import concourse.bacc as bacc
import concourse.bass as bass
import concourse.mybir as mybir
import concourse.tile as tile
from concourse import bass_utils, dve_ops
from concourse.dve_spec import Spec, Src0, Src1, scan, AluOp, lower, _has_src1
from concourse.dve_uop import DveOpSpec

N = 50000
E = 150000
D_IN = 42
D_H = 42
E_IN = 10
E_H = 128
STEPS = 6
N_CORES = 8
NPC = N // N_CORES          # 6250 nodes per core
WIN = 128                   # scatter window (node block) size
N_WIN = (NPC + WIN - 1) // WIN  # 49 windows per core, last partial (106)
NW = D_H * D_H              # 1764
F32 = mybir.dt.float32
BF16 = mybir.dt.bfloat16
I32 = mybir.dt.int32
BF = ml_dtypes.bfloat16

# chunked all-gather: windows split in 3, collective per chunk as it completes.
# cc_out rows are chunk-major: [chunk][core][local row within chunk].
C_CC = 3
WB = [0, 16, 33, N_WIN]                      # window bounds per chunk
RB = [0, WB[1] * WIN, WB[2] * WIN, NPC]      # node-row bounds per chunk
ROWS_C = [RB[c + 1] - RB[c] for c in range(C_CC)]
BASE_C = [0, N_CORES * ROWS_C[0], N_CORES * (ROWS_C[0] + ROWS_C[1])]


def _remap_rows(v):
    """Map global node id -> row in the chunk-major cc_out layout."""
    r = v // NPC
    l = v - r * NPC
    c = (l >= RB[1]).astype(np.int64) + (l >= RB[2]).astype(np.int64)
    rows = np.asarray(ROWS_C)[c]
    base = np.asarray(BASE_C)[c]
    rb = np.asarray(RB)[c]
    return (base + r * rows + (l - rb)).astype(np.int32)


def _register_prefix_mac():
    name = "PREFIX_MAC_GNN"
    if name in dve_ops._SUB_OPCODE_FOR_NAME:
        return next(op for op in dve_ops.OPS if op.name == name)
    spec = Spec(
        body=scan(AluOp.ADD, Src0 * Src1),
        reference=lambda in0, in1, s0, s1, imm2: np.cumsum(
            (in0.astype(np.float32) * in1).reshape(in0.shape[0], -1), axis=-1
        ),
    )
    shas = {}
    row = dve_ops._CUSTOM_DVE_ROW_BASE + len(dve_ops.OPS)
    for ver in ("v3", "v4"):
        uops = lower(spec, ver=ver)
        shas[ver] = DveOpSpec(name=name, opcode=row, uops=uops,
                              rd1_en=_has_src1(spec)).sha(ver)
    op = dve_ops.DveOp(name, spec, subdim=False, uops_sha=shas)
    dve_ops.OPS.append(op)
    dve_ops._SUB_OPCODE_FOR_NAME[name] = row
    dve_ops.CUSTOM_DVE_SPECS[name] = spec
    return op


def _host_prep(n_feat, e_feat, src, dst):
    """Sort edges by dst, shard by dst block, pad each (core, window) edge run
    onto a shared slot grid so the tile->window map is identical on all cores."""
    order = np.argsort(dst, kind="stable")
    src_s, dst_s, ef_s = src[order], dst[order], e_feat[order]

    # per (core, window) counts
    core_e = dst_s // NPC
    loc = dst_s - core_e * NPC
    win_e = loc // WIN
    cnt = np.zeros((N_CORES, N_WIN), dtype=np.int64)
    np.add.at(cnt, (core_e, win_e), 1)

    slot_cnt = cnt.max(axis=0)                       # shared grid
    G = np.concatenate([[0], np.cumsum(slot_cnt)])   # window slot boundaries
    total = int(G[-1])
    T = (total + 127) // 128                         # edge tiles per core
    E_PAD = T * 128

    # per-core padded edge arrays
    src_pad = np.zeros((N_CORES, E_PAD), dtype=np.int32)
    dstrel_pad = np.full((N_CORES, E_PAD), -1.0, dtype=np.float32)
    ef_pad = np.zeros((N_CORES, E_PAD, E_IN), dtype=np.float32)

    # tile -> window band
    w0 = np.zeros(T, dtype=np.int64)       # first window overlapping tile t
    bw = np.zeros(T, dtype=np.int64)       # how many windows overlap tile t
    for t in range(T):
        lo, hi = t * 128, min((t + 1) * 128, total)
        wlo = int(np.searchsorted(G, lo, side="right") - 1)
        whi = int(np.searchsorted(G, max(hi - 1, lo), side="right") - 1)
        wlo, whi = min(wlo, N_WIN - 1), min(whi, N_WIN - 1)
        w0[t] = wlo
        bw[t] = whi - wlo + 1
    B_W = int(bw.max())

    # fill padded arrays: window w of core k occupies slots [G[w], G[w]+cnt[k,w])
    core_starts = np.searchsorted(core_e, np.arange(N_CORES))
    for k in range(N_CORES):
        base = core_starts[k]
        cw = np.concatenate([[0], np.cumsum(cnt[k])])
        for w in range(N_WIN):
            s0, s1 = int(base + cw[w]), int(base + cw[w + 1])
            g0 = int(G[w])
            n_e = s1 - s0
            src_pad[k, g0:g0 + n_e] = src_s[s0:s1]
            ef_pad[k, g0:g0 + n_e] = ef_s[s0:s1]
            # dst_rel relative to the band anchor of the edge's tile
            slots = np.arange(g0, g0 + n_e)
            dstrel_pad[k, g0:g0 + n_e] = (
                loc[s0:s1] - w0[slots // 128] * WIN).astype(np.float32)

    # scatter pair list (t, w) from actual overlap, and per-window tile ranges
    pairs = []
    for t in range(T):
        for j in range(int(bw[t])):
            w = int(w0[t]) + j
            if w < N_WIN:
                pairs.append((t, w))
    win_tiles = {w: [t for (t, ww) in pairs if ww == w] for w in range(N_WIN)}

    grid = dict(T=T, E_PAD=E_PAD, B_W=B_W, w0=w0, bw=bw, win_tiles=win_tiles)

    per_core = []
    for k in range(N_CORES):
        per_core.append(dict(
            e_featT=np.ascontiguousarray(ef_pad[k].T).astype(BF),  # [10, E_PAD]
            n_featT=np.ascontiguousarray(n_feat[k * NPC:(k + 1) * NPC].T),  # [42, NPC]
            src_idx=np.ascontiguousarray(src_pad[k].reshape(T, 128).T).astype(np.int32),  # [128, T]
            dst_rel=np.ascontiguousarray(dstrel_pad[k].reshape(T, 128).T),  # [128, T]
        ))
    return grid, per_core


def _build_program(grid):
    T, B_W = grid["T"], grid["B_W"]
    w0, bw, win_tiles = grid["w0"], grid["bw"], grid["win_tiles"]
    PREFIX_MAC = _register_prefix_mac()

    # per-tile column offsets into the resident one-hot bank
    oh_off = np.zeros(T + 1, dtype=np.int64)
    for t in range(T):
        oh_off[t + 1] = oh_off[t] + int(bw[t]) * WIN
    OH_COLS = int(oh_off[T])

    nc = bacc.Bacc("TRN2", target_bir_lowering=False, debug=False,
                   num_devices=N_CORES)

    # ---- kernel I/O ----
    e_featT = nc.dram_tensor("e_featT", [E_IN, grid["E_PAD"]], BF16, kind="ExternalInput")
    n_featT = nc.dram_tensor("n_featT", [D_IN, NPC], F32, kind="ExternalInput")
    src_idx = nc.dram_tensor("src_idx", [128, T], I32, kind="ExternalInput")
    dst_rel = nc.dram_tensor("dst_rel", [128, T], F32, kind="ExternalInput")
    iota = nc.dram_tensor("iota", [128, B_W * WIN], BF16, kind="ExternalInput")
    en1_w = nc.dram_tensor("en1_w", [E_IN, E_H], BF16, kind="ExternalInput")
    en1_bc = nc.dram_tensor("en1_bc", [E_H, 1], F32, kind="ExternalInput")
    en2_wp = nc.dram_tensor("en2_wp", [E_H, NW], BF16, kind="ExternalInput")
    en2_bp = nc.dram_tensor("en2_bp", [1, NW], BF16, kind="ExternalInput")
    lin0_wt = nc.dram_tensor("lin0_wt", [D_IN, D_H], F32, kind="ExternalInput")
    lin0_bc = nc.dram_tensor("lin0_bc", [D_H, 1], F32, kind="ExternalInput")
    msgw_top = nc.dram_tensor("msgw_top", [D_H, D_H], F32, kind="ExternalInput")
    msgw_bot = nc.dram_tensor("msgw_bot", [D_H, D_H], F32, kind="ExternalInput")
    msgb_c = nc.dram_tensor("msgb_c", [D_H, 1], F32, kind="ExternalInput")
    convb_c = nc.dram_tensor("convb_c", [D_H, 1], F32, kind="ExternalInput")
    ident = nc.dram_tensor("ident", [D_H, D_H], F32, kind="ExternalInput")
    ones_bf = nc.dram_tensor("ones_bf", [1, 128], BF16, kind="ExternalInput")
    y = nc.dram_tensor("y", [NPC, D_H], F32, kind="ExternalOutput")

    with tile.TileContext(nc) as tc:
        with (
            tc.tile_pool(name="const", bufs=1) as cpool,
            tc.tile_pool(name="dram", bufs=1, space="DRAM") as dram,
        ):
            # ---- persistent SBUF residents ----
            nfT_sb = cpool.tile([D_IN, NPC], F32)
            srci_sb = cpool.tile([128, T], I32)
            dstr_sb = cpool.tile([128, T], F32)
            iota_sb = cpool.tile([128, B_W * WIN], BF16)
            en1w_sb = cpool.tile([E_IN, E_H], BF16)
            en1bc_sb = cpool.tile([E_H, 1], F32)
            en2wp_sb = cpool.tile([E_H, NW], BF16)
            en2bp_sb = cpool.tile([1, NW], BF16)
            lin0w_sb = cpool.tile([D_IN, D_H], F32)
            lin0bc_sb = cpool.tile([D_H, 1], F32)
            mwt_sb = cpool.tile([D_H, D_H], F32)
            mwb_sb = cpool.tile([D_H, D_H], F32)
            mbc_sb = cpool.tile([D_H, 1], F32)
            cvbc_sb = cpool.tile([D_H, 1], F32)
            id_sb = cpool.tile([D_H, D_H], F32)
            onesbf_sb = cpool.tile([1, 128], BF16)
            outT_a = cpool.tile([D_H, NPC], F32)
            outT_b = cpool.tile([D_H, NPC], F32)
            pfx = cpool.tile([128, 1 + NW], F32)
            oh_all = cpool.tile([128, OH_COLS], BF16)

            for sb, dr in [(nfT_sb, n_featT), (srci_sb, src_idx),
                           (dstr_sb, dst_rel), (iota_sb, iota), (en1w_sb, en1_w),
                           (en1bc_sb, en1_bc), (en2wp_sb, en2_wp), (en2bp_sb, en2_bp),
                           (lin0w_sb, lin0_wt), (lin0bc_sb, lin0_bc), (mwt_sb, msgw_top),
                           (mwb_sb, msgw_bot), (mbc_sb, msgb_c), (cvbc_sb, convb_c),
                           (id_sb, ident), (onesbf_sb, ones_bf)]:
                nc.sync.dma_start(sb[:], dr[:])
            nc.gpsimd.memset(pfx[:, 0:1], 0.0)

            # ---- DRAM scratch ----
            w_dram = dram.tile([T * 128, NW], BF16)
            cc_in = [dram.tile([NPC, D_H], BF16, name=f"cc_in{i}") for i in range(2)]
            cc_out = [dram.tile([N, D_H], BF16, name=f"cc_out{i}", addr_space="Shared")
                      for i in range(STEPS)]

            # =========== setup: build W' (bf16) in HBM ===========
            ECH = 16  # e_feat tiles per SBUF chunk
            with (
                tc.tile_pool(name="su_h", bufs=1) as su_h,
                tc.tile_pool(name="su_sb", bufs=2) as su_sb,
                tc.tile_pool(name="su_e", bufs=2) as su_e,
                tc.tile_pool(name="su_ph", bufs=2, space="PSUM") as su_ph,
                tc.tile_pool(name="su_pw", bufs=3, space="PSUM") as su_pw,
            ):
                # phase 0: h_all = relu(e_feat @ en1_w + b) for every edge tile,
                # and the step-invariant one-hot scatter bank on the idle DVE
                h_all = su_h.tile([128, T * 128], BF16)
                e_ch = None
                for t in range(T):
                    if t % ECH == 0:
                        c0 = t * 128
                        c1 = min((t + ECH) * 128, grid["E_PAD"])
                        e_ch = su_e.tile([E_IN, ECH * 128], BF16, name="e_ch")
                        nc.sync.dma_start(e_ch[:, :c1 - c0], e_featT[:, c0:c1])
                    ph = su_ph.tile([128, 128], F32, name="ph")
                    o = (t % ECH) * 128
                    nc.tensor.matmul(ph[:], lhsT=en1w_sb[:], rhs=e_ch[:, o:o + 128],
                                     start=True, stop=True)
                    nc.scalar.activation(h_all[:, t * 128:(t + 1) * 128], ph[:],
                                         mybir.ActivationFunctionType.Relu,
                                         bias=en1bc_sb[:, 0:1])
                    bwt = int(bw[t])
                    nc.vector.tensor_scalar(
                        out=oh_all[:, int(oh_off[t]):int(oh_off[t]) + bwt * WIN],
                        in0=iota_sb[:, :bwt * WIN],
                        scalar1=dstr_sb[:, t:t + 1],
                        scalar2=None, op0=mybir.AluOpType.is_equal)

                # bias broadcast (chunks 0-2; chunk 3 gets bias via PE)
                bias_sb = su_sb.tile([128, 1536], BF16, name="bias_sb")
                for j in range(3):
                    o0 = j * 512
                    pb = su_pw.tile([128, 512], F32, name="pb")
                    nc.tensor.matmul(pb[:], lhsT=onesbf_sb[:1, :],
                                     rhs=en2bp_sb[:, o0:o0 + 512],
                                     start=True, stop=True)
                    nc.vector.tensor_scalar_add(out=bias_sb[:, o0:o0 + 512],
                                                in0=pb[:], scalar1=0.0)

                nsz = [512, 512, 512, NW - 3 * 512]
                for t in range(T):
                    h_t = h_all[:, t * 128:(t + 1) * 128]
                    w_sb = su_sb.tile([128, NW], BF16, name="w_sb")
                    for j in range(4):
                        o0 = j * 512
                        pw = su_pw.tile([128, 512], F32, name="pw")
                        if j < 3:
                            # bias added on DVE together with the bf16 cast
                            nc.tensor.matmul(pw[:, :nsz[j]], lhsT=h_t,
                                             rhs=en2wp_sb[:, o0:o0 + nsz[j]],
                                             start=True, stop=True)
                            nc.vector.tensor_tensor(
                                out=w_sb[:, o0:o0 + nsz[j]], in0=pw[:, :nsz[j]],
                                in1=bias_sb[:, o0:o0 + nsz[j]],
                                op=mybir.AluOpType.add)
                        else:
                            # bias via K=1 matmul; cast on Scalar
                            nc.tensor.matmul(pw[:, :nsz[j]], lhsT=onesbf_sb[:1, :],
                                             rhs=en2bp_sb[:, o0:o0 + nsz[j]],
                                             start=True, stop=False)
                            nc.tensor.matmul(pw[:, :nsz[j]], lhsT=h_t,
                                             rhs=en2wp_sb[:, o0:o0 + nsz[j]],
                                             start=False, stop=True)
                            nc.scalar.copy(w_sb[:, o0:o0 + nsz[j]],
                                           pw[:, :nsz[j]])
                    nc.sync.dma_start(w_dram[t * 128:(t + 1) * 128, :], w_sb[:])

            # =========== step pools ===========
            with (
                tc.tile_pool(name="st_w", bufs=6) as p_w,
                tc.tile_pool(name="st_x", bufs=8) as p_x,
                tc.tile_pool(name="st_m", bufs=4) as p_m,
                tc.tile_pool(name="st_sm", bufs=4) as p_sm,
                tc.tile_pool(name="ps_ag", bufs=3, space="PSUM") as ps_ag,
                tc.tile_pool(name="ps_up", bufs=2, space="PSUM") as ps_up,
                tc.tile_pool(name="ps_tr", bufs=1, space="PSUM") as ps_tr,
            ):
                def window_cols(w):
                    n0 = w * WIN
                    m = min(WIN, NPC - n0)
                    return n0, m

                def chunk_of(w):
                    return (w >= WB[1]) + (w >= WB[2])

                def update_window(w, outT_cur, outT_new, aggr_ps, step):
                    """Window epilogue: finish aggr, relu, update matmul, transpose, DMA."""
                    n0, m = window_cols(w)
                    last = step == STEPS
                    # + out (identity residual into conv); conv bias folds into relu
                    nc.tensor.matmul(aggr_ps[:, :m], lhsT=id_sb[:],
                                     rhs=outT_cur[:, n0:n0 + m], start=False, stop=True)
                    mT_sb = p_sm.tile([D_H, WIN], F32, name="mT_sb")
                    nc.scalar.activation(mT_sb[:, :m], aggr_ps[:, :m],
                                         mybir.ActivationFunctionType.Relu,
                                         bias=cvbc_sb[:, 0:1])
                    up = ps_up.tile([D_H, WIN], F32, name="up")
                    nc.tensor.matmul(up[:, :m], lhsT=mwt_sb[:], rhs=mT_sb[:, :m],
                                     start=True, stop=False)
                    nc.tensor.matmul(up[:, :m], lhsT=mwb_sb[:], rhs=outT_cur[:, n0:n0 + m],
                                     start=False, stop=not last)
                    if last:
                        nc.tensor.matmul(up[:, :m], lhsT=id_sb[:], rhs=nfT_sb[:, n0:n0 + m],
                                         start=False, stop=True)
                    # msg bias folds into the PSUM->SBUF copy
                    nc.scalar.activation(outT_new[:, n0:n0 + m], up[:, :m],
                                         mybir.ActivationFunctionType.Identity,
                                         bias=mbc_sb[:, 0:1])
                    tr = ps_tr.tile([128, D_H], F32, name="tr")
                    nc.tensor.transpose(tr[:m, :], outT_new[:, n0:n0 + m], id_sb[:])
                    if last:
                        rows = p_sm.tile([128, D_H], F32, name="rows_f")
                        nc.scalar.copy(rows[:m, :], tr[:m, :])
                        nc.sync.dma_start(y[n0:n0 + m, :], rows[:m, :])
                    else:
                        rows = p_sm.tile([128, D_H], BF16, name="rows_b")
                        nc.scalar.copy(rows[:m, :], tr[:m, :])
                        nc.sync.dma_start(cc_in[step % 2][n0:n0 + m, :],
                                          rows[:m, :])

                def all_gather(step):
                    nc.gpsimd.collective_compute(
                        "AllGather", mybir.AluOpType.bypass,
                        replica_groups=[list(range(N_CORES))],
                        ins=[cc_in[step % 2].opt()], outs=[cc_out[step].opt()])

                # =========== lin0: out0 = relu(n_feat @ lin0_w + b) ===========
                for w in range(N_WIN):
                    n0, m = window_cols(w)
                    up = ps_up.tile([D_H, WIN], F32, name="up")
                    nc.tensor.matmul(up[:, :m], lhsT=lin0w_sb[:], rhs=nfT_sb[:, n0:n0 + m],
                                     start=True, stop=True)
                    nc.scalar.activation(outT_a[:, n0:n0 + m], up[:, :m],
                                         mybir.ActivationFunctionType.Relu,
                                         bias=lin0bc_sb[:, 0:1])
                    tr = ps_tr.tile([128, D_H], F32, name="tr")
                    nc.tensor.transpose(tr[:m, :], outT_a[:, n0:n0 + m], id_sb[:])
                    rows = p_sm.tile([128, D_H], BF16, name="rows_b")
                    nc.scalar.copy(rows[:m, :], tr[:m, :])
                    nc.sync.dma_start(cc_in[0][n0:n0 + m, :], rows[:m, :])
                all_gather(0)

                # =========== message passing steps ===========
                for step in range(1, STEPS + 1):
                    outT_cur = outT_a if step % 2 == 1 else outT_b
                    outT_new = outT_b if step % 2 == 1 else outT_a
                    src_buf = cc_out[step - 1]
                    aggr_of = {}
                    for t in range(T):
                        x_g = p_x.tile([128, D_H], BF16, name="x_g")
                        nc.gpsimd.indirect_dma_start(
                            out=x_g[:], out_offset=None, in_=src_buf[:],
                            in_offset=bass.IndirectOffsetOnAxis(
                                ap=srci_sb[:, t:t + 1], axis=0))
                        w_t = p_w.tile([128, NW], BF16, name="w_t")
                        nc.sync.dma_start(w_t[:], w_dram[t * 128:(t + 1) * 128, :])
                        nc.vector._custom_dve(
                            PREFIX_MAC, out=pfx[:, 1:1 + NW], in0=w_t[:],
                            in1=x_g[:, None, :].to_broadcast([128, D_H, D_H]))
                        msg = p_m.tile([128, D_H], BF16, name="msg")
                        nc.vector.tensor_tensor(
                            out=msg[:], in0=pfx[:, D_H:1 + NW:D_H],
                            in1=pfx[:, 0:NW:D_H], op=mybir.AluOpType.subtract)
                        bwt = int(bw[t])
                        # scatter matmuls against the resident one-hot bank
                        for j in range(bwt):
                            w = int(w0[t]) + j
                            if w >= N_WIN:
                                continue
                            tiles_w = win_tiles[w]
                            if w not in aggr_of:
                                aggr_of[w] = ps_ag.tile([D_H, WIN], F32, name="aggr")
                            first = t == tiles_w[0]
                            last_t = t == tiles_w[-1]
                            o_c = int(oh_off[t]) + j * WIN
                            nc.tensor.matmul(aggr_of[w][:], lhsT=msg[:],
                                             rhs=oh_all[:, o_c:o_c + WIN],
                                             start=first, stop=False)
                            if last_t:
                                update_window(w, outT_cur, outT_new,
                                              aggr_of.pop(w), step)
                    if step < STEPS:
                        all_gather(step)

    nc.compile()
    return nc


_CACHED = {}


def kernel(n_feat, e_feat, src, dst, lin0_w, lin0_b, en1_w, en1_b,
           en2_w, en2_b, conv_bias, msg_w, msg_b):
    n_feat = np.asarray(n_feat, dtype=np.float32)
    e_feat = np.asarray(e_feat, dtype=np.float32)
    src = np.asarray(src, dtype=np.int32)
    dst = np.asarray(dst, dtype=np.int32)

    grid, per_core = _host_prep(n_feat, e_feat, src, dst)

    key = (grid["T"], grid["B_W"], tuple(grid["w0"].tolist()))
    if key not in _CACHED:
        _CACHED.clear()
        _CACHED[key] = _build_program(grid)
    nc = _CACHED[key]

    en2_wp = np.ascontiguousarray(
        np.asarray(en2_w, np.float32).reshape(E_H, D_H, D_H).transpose(0, 2, 1).reshape(E_H, NW))
    shared = dict(
        iota=np.tile(np.arange(grid["B_W"] * WIN, dtype=np.float32), (128, 1)).astype(BF),
        en1_w=np.asarray(en1_w, np.float32).astype(BF),
        en1_bc=np.asarray(en1_b, np.float32).reshape(E_H, 1),
        en2_wp=en2_wp.astype(BF),
        en2_bp=np.ascontiguousarray(
            np.asarray(en2_b, np.float32).reshape(D_H, D_H).T.reshape(1, NW)).astype(BF),
        lin0_wt=np.asarray(lin0_w, np.float32),
        lin0_bc=np.asarray(lin0_b, np.float32).reshape(D_H, 1),
        msgw_top=np.ascontiguousarray(np.asarray(msg_w, np.float32)[:D_H, :]),
        msgw_bot=np.ascontiguousarray(np.asarray(msg_w, np.float32)[D_H:, :]),
        msgb_c=np.asarray(msg_b, np.float32).reshape(D_H, 1),
        convb_c=np.asarray(conv_bias, np.float32).reshape(D_H, 1),
        ident=np.eye(D_H, dtype=np.float32),
        ones_bf=np.ones((1, 128), dtype=BF),
    )
    in_maps = []
    for k in range(N_CORES):
        m = dict(shared)
        m.update(per_core[k])
        in_maps.append(m)

    res = bass_utils.run_bass_kernel_spmd(nc, in_maps, core_ids=list(range(N_CORES)))
    out = np.concatenate([res.results[k]["y"] for k in range(N_CORES)], axis=0)
    return out.astype(np.float32)


# revision 35
# speedup vs baseline: 1.4801x; 1.0111x over previous
"""Trainium2 Bass kernel for nn_GatherModel (NNConv GNN message passing).

8-core SPMD, edge-parallel sharded by destination node block:
  - core k owns nodes [k*6250, (k+1)*6250) and all edges whose dst lies there
  - per-edge weight matrices W'_e (o-major) are built once on device (PE,
    bf16) and streamed bf16 from HBM each of the 6 message-passing steps
  - per-edge contraction msg = x_src @ W_e runs on the Vector engine via a
    custom fused multiply+prefix-scan DVE op (bf16 in, fp32 out),
    extracting per-o sums by strided prefix differences
  - scatter (segment-sum over dst) is a PE matmul against on-device-built
    bf16 one-hot window matrices; node update runs fp32 in transposed
    feature layout
  - each step ends with an 8-core AllGather of bf16 node features
"""
import numpy as np
import ml_dtypes

import concourse.bacc as bacc
import concourse.bass as bass
import concourse.mybir as mybir
import concourse.tile as tile
from concourse import bass_utils, dve_ops
from concourse.dve_spec import Spec, Src0, Src1, scan, AluOp, lower, _has_src1
from concourse.dve_uop import DveOpSpec

N = 50000
E = 150000
D_IN = 42
D_H = 42
E_IN = 10
E_H = 128
STEPS = 6
N_CORES = 8
NPC = N // N_CORES          # 6250 nodes per core
WIN = 128                   # scatter window (node block) size
N_WIN = (NPC + WIN - 1) // WIN  # 49 windows per core, last partial (106)
NW = D_H * D_H              # 1764
F32 = mybir.dt.float32
BF16 = mybir.dt.bfloat16
I32 = mybir.dt.int32
BF = ml_dtypes.bfloat16

# chunked all-gather: windows split in 3, collective per chunk as it completes.
# cc_out rows are chunk-major: [chunk][core][local row within chunk].
C_CC = 3
WB = [0, 16, 33, N_WIN]                      # window bounds per chunk
RB = [0, WB[1] * WIN, WB[2] * WIN, NPC]      # node-row bounds per chunk
ROWS_C = [RB[c + 1] - RB[c] for c in range(C_CC)]
BASE_C = [0, N_CORES * ROWS_C[0], N_CORES * (ROWS_C[0] + ROWS_C[1])]


def _remap_rows(v):
    """Map global node id -> row in the chunk-major cc_out layout."""
    r = v // NPC
    l = v - r * NPC
    c = (l >= RB[1]).astype(np.int64) + (l >= RB[2]).astype(np.int64)
    rows = np.asarray(ROWS_C)[c]
    base = np.asarray(BASE_C)[c]
    rb = np.asarray(RB)[c]
    return (base + r * rows + (l - rb)).astype(np.int32)


def _register_prefix_mac():
    name = "PREFIX_MAC_GNN"
    if name in dve_ops._SUB_OPCODE_FOR_NAME:
        return next(op for op in dve_ops.OPS if op.name == name)
    spec = Spec(
        body=scan(AluOp.ADD, Src0 * Src1),
        reference=lambda in0, in1, s0, s1, imm2: np.cumsum(
            (in0.astype(np.float32) * in1).reshape(in0.shape[0], -1), axis=-1
        ),
    )
    shas = {}
    row = dve_ops._CUSTOM_DVE_ROW_BASE + len(dve_ops.OPS)
    for ver in ("v3", "v4"):
        uops = lower(spec, ver=ver)
        shas[ver] = DveOpSpec(name=name, opcode=row, uops=uops,
                              rd1_en=_has_src1(spec)).sha(ver)
    op = dve_ops.DveOp(name, spec, subdim=False, uops_sha=shas)
    dve_ops.OPS.append(op)
    dve_ops._SUB_OPCODE_FOR_NAME[name] = row
    dve_ops.CUSTOM_DVE_SPECS[name] = spec
    return op


def _host_prep(n_feat, e_feat, src, dst):
    """Sort edges by dst, shard by dst block, pad each (core, window) edge run
    onto a shared slot grid so the tile->window map is identical on all cores."""
    order = np.argsort(dst, kind="stable")
    src_s, dst_s, ef_s = src[order], dst[order], e_feat[order]

    # per (core, window) counts
    core_e = dst_s // NPC
    loc = dst_s - core_e * NPC
    win_e = loc // WIN
    cnt = np.zeros((N_CORES, N_WIN), dtype=np.int64)
    np.add.at(cnt, (core_e, win_e), 1)

    slot_cnt = cnt.max(axis=0)                       # shared grid
    G = np.concatenate([[0], np.cumsum(slot_cnt)])   # window slot boundaries
    total = int(G[-1])
    T = (total + 127) // 128                         # edge tiles per core
    E_PAD = T * 128

    # per-core padded edge arrays
    src_pad = np.zeros((N_CORES, E_PAD), dtype=np.int32)
    dstrel_pad = np.full((N_CORES, E_PAD), -1.0, dtype=np.float32)
    ef_pad = np.zeros((N_CORES, E_PAD, E_IN), dtype=np.float32)

    # tile -> window band
    w0 = np.zeros(T, dtype=np.int64)       # first window overlapping tile t
    bw = np.zeros(T, dtype=np.int64)       # how many windows overlap tile t
    for t in range(T):
        lo, hi = t * 128, min((t + 1) * 128, total)
        wlo = int(np.searchsorted(G, lo, side="right") - 1)
        whi = int(np.searchsorted(G, max(hi - 1, lo), side="right") - 1)
        wlo, whi = min(wlo, N_WIN - 1), min(whi, N_WIN - 1)
        w0[t] = wlo
        bw[t] = whi - wlo + 1
    B_W = int(bw.max())

    # fill padded arrays: window w of core k occupies slots [G[w], G[w]+cnt[k,w])
    core_starts = np.searchsorted(core_e, np.arange(N_CORES))
    for k in range(N_CORES):
        base = core_starts[k]
        cw = np.concatenate([[0], np.cumsum(cnt[k])])
        for w in range(N_WIN):
            s0, s1 = int(base + cw[w]), int(base + cw[w + 1])
            g0 = int(G[w])
            n_e = s1 - s0
            src_pad[k, g0:g0 + n_e] = src_s[s0:s1]
            ef_pad[k, g0:g0 + n_e] = ef_s[s0:s1]
            # dst_rel relative to the band anchor of the edge's tile
            slots = np.arange(g0, g0 + n_e)
            dstrel_pad[k, g0:g0 + n_e] = (
                loc[s0:s1] - w0[slots // 128] * WIN).astype(np.float32)

    # scatter pair list (t, w) from actual overlap, and per-window tile ranges
    pairs = []
    for t in range(T):
        for j in range(int(bw[t])):
            w = int(w0[t]) + j
            if w < N_WIN:
                pairs.append((t, w))
    win_tiles = {w: [t for (t, ww) in pairs if ww == w] for w in range(N_WIN)}

    grid = dict(T=T, E_PAD=E_PAD, B_W=B_W, w0=w0, bw=bw, win_tiles=win_tiles)

    per_core = []
    for k in range(N_CORES):
        per_core.append(dict(
            e_featT=np.ascontiguousarray(ef_pad[k].T).astype(BF),  # [10, E_PAD]
            n_featT=np.ascontiguousarray(n_feat[k * NPC:(k + 1) * NPC].T),  # [42, NPC]
            src_idx=np.ascontiguousarray(src_pad[k].reshape(T, 128).T).astype(np.int32),  # [128, T]
            dst_rel=np.ascontiguousarray(dstrel_pad[k].reshape(T, 128).T),  # [128, T]
        ))
    return grid, per_core


def _build_program(grid):
    T, B_W = grid["T"], grid["B_W"]
    w0, bw, win_tiles = grid["w0"], grid["bw"], grid["win_tiles"]
    PREFIX_MAC = _register_prefix_mac()

    # per-tile column offsets into the resident one-hot bank
    oh_off = np.zeros(T + 1, dtype=np.int64)
    for t in range(T):
        oh_off[t + 1] = oh_off[t] + int(bw[t]) * WIN
    OH_COLS = int(oh_off[T])

    nc = bacc.Bacc("TRN2", target_bir_lowering=False, debug=False,
                   num_devices=N_CORES)

    # ---- kernel I/O ----
    e_featT = nc.dram_tensor("e_featT", [E_IN, grid["E_PAD"]], BF16, kind="ExternalInput")
    n_featT = nc.dram_tensor("n_featT", [D_IN, NPC], F32, kind="ExternalInput")
    src_idx = nc.dram_tensor("src_idx", [128, T], I32, kind="ExternalInput")
    dst_rel = nc.dram_tensor("dst_rel", [128, T], F32, kind="ExternalInput")
    iota = nc.dram_tensor("iota", [128, B_W * WIN], BF16, kind="ExternalInput")
    en1_w = nc.dram_tensor("en1_w", [E_IN, E_H], BF16, kind="ExternalInput")
    en1_bc = nc.dram_tensor("en1_bc", [E_H, 1], F32, kind="ExternalInput")
    en2_wp = nc.dram_tensor("en2_wp", [E_H, NW], BF16, kind="ExternalInput")
    en2_bp = nc.dram_tensor("en2_bp", [1, NW], BF16, kind="ExternalInput")
    lin0_wt = nc.dram_tensor("lin0_wt", [D_IN, D_H], F32, kind="ExternalInput")
    lin0_bc = nc.dram_tensor("lin0_bc", [D_H, 1], F32, kind="ExternalInput")
    msgw_top = nc.dram_tensor("msgw_top", [D_H, D_H], F32, kind="ExternalInput")
    msgw_bot = nc.dram_tensor("msgw_bot", [D_H, D_H], F32, kind="ExternalInput")
    msgb_c = nc.dram_tensor("msgb_c", [D_H, 1], F32, kind="ExternalInput")
    convb_c = nc.dram_tensor("convb_c", [D_H, 1], F32, kind="ExternalInput")
    ident = nc.dram_tensor("ident", [D_H, D_H], F32, kind="ExternalInput")
    ones_bf = nc.dram_tensor("ones_bf", [1, 128], BF16, kind="ExternalInput")
    y = nc.dram_tensor("y", [NPC, D_H], F32, kind="ExternalOutput")

    with tile.TileContext(nc) as tc:
        with (
            tc.tile_pool(name="const", bufs=1) as cpool,
            tc.tile_pool(name="dram", bufs=1, space="DRAM") as dram,
        ):
            # ---- persistent SBUF residents ----
            nfT_sb = cpool.tile([D_IN, NPC], F32)
            srci_sb = cpool.tile([128, T], I32)
            dstr_sb = cpool.tile([128, T], F32)
            iota_sb = cpool.tile([128, B_W * WIN], BF16)
            en1w_sb = cpool.tile([E_IN, E_H], BF16)
            en1bc_sb = cpool.tile([E_H, 1], F32)
            en2wp_sb = cpool.tile([E_H, NW], BF16)
            en2bp_sb = cpool.tile([1, NW], BF16)
            lin0w_sb = cpool.tile([D_IN, D_H], F32)
            lin0bc_sb = cpool.tile([D_H, 1], F32)
            mwt_sb = cpool.tile([D_H, D_H], F32)
            mwb_sb = cpool.tile([D_H, D_H], F32)
            mbc_sb = cpool.tile([D_H, 1], F32)
            cvbc_sb = cpool.tile([D_H, 1], F32)
            id_sb = cpool.tile([D_H, D_H], F32)
            onesbf_sb = cpool.tile([1, 128], BF16)
            outT_a = cpool.tile([D_H, NPC], F32)
            outT_b = cpool.tile([D_H, NPC], F32)
            pfx = cpool.tile([128, 1 + NW], F32)
            oh_all = cpool.tile([128, OH_COLS], BF16)

            for sb, dr in [(nfT_sb, n_featT), (srci_sb, src_idx),
                           (dstr_sb, dst_rel), (iota_sb, iota), (en1w_sb, en1_w),
                           (en1bc_sb, en1_bc), (en2wp_sb, en2_wp), (en2bp_sb, en2_bp),
                           (lin0w_sb, lin0_wt), (lin0bc_sb, lin0_bc), (mwt_sb, msgw_top),
                           (mwb_sb, msgw_bot), (mbc_sb, msgb_c), (cvbc_sb, convb_c),
                           (id_sb, ident), (onesbf_sb, ones_bf)]:
                nc.sync.dma_start(sb[:], dr[:])
            nc.gpsimd.memset(pfx[:, 0:1], 0.0)

            # ---- DRAM scratch ----
            w_dram = dram.tile([T * 128, NW], BF16)
            cc_in = [dram.tile([NPC, D_H], BF16, name=f"cc_in{i}") for i in range(2)]
            cc_out = [dram.tile([N, D_H], BF16, name=f"cc_out{i}", addr_space="Shared")
                      for i in range(STEPS)]

            # =========== setup: build W' (bf16) in HBM ===========
            ECH = 16  # e_feat tiles per SBUF chunk
            with (
                tc.tile_pool(name="su_h", bufs=1) as su_h,
                tc.tile_pool(name="su_sb", bufs=2) as su_sb,
                tc.tile_pool(name="su_e", bufs=2) as su_e,
                tc.tile_pool(name="su_ph", bufs=2, space="PSUM") as su_ph,
                tc.tile_pool(name="su_pw", bufs=3, space="PSUM") as su_pw,
            ):
                # bias broadcast first (chunks 0-2; chunk 3 gets bias via PE)
                bias_sb = su_sb.tile([128, 1536], BF16, name="bias_sb")
                for j in range(3):
                    o0 = j * 512
                    pb = su_pw.tile([128, 512], F32, name="pb")
                    nc.tensor.matmul(pb[:], lhsT=onesbf_sb[:1, :],
                                     rhs=en2bp_sb[:, o0:o0 + 512],
                                     start=True, stop=True)
                    nc.vector.tensor_scalar_add(out=bias_sb[:, o0:o0 + 512],
                                                in0=pb[:], scalar1=0.0)

                # one fused loop: h tile, one-hot bank, then the W' build for
                # the same tile — keeps every engine queue fed from the start
                h_all = su_h.tile([128, T * 128], BF16)
                nsz = [512, 512, 512, NW - 3 * 512]
                e_ch = None
                for t in range(T):
                    if t % ECH == 0:
                        c0 = t * 128
                        c1 = min((t + ECH) * 128, grid["E_PAD"])
                        e_ch = su_e.tile([E_IN, ECH * 128], BF16, name="e_ch")
                        nc.sync.dma_start(e_ch[:, :c1 - c0], e_featT[:, c0:c1])
                    ph = su_ph.tile([128, 128], F32, name="ph")
                    o = (t % ECH) * 128
                    nc.tensor.matmul(ph[:], lhsT=en1w_sb[:], rhs=e_ch[:, o:o + 128],
                                     start=True, stop=True)
                    h_t = h_all[:, t * 128:(t + 1) * 128]
                    nc.scalar.activation(h_t, ph[:],
                                         mybir.ActivationFunctionType.Relu,
                                         bias=en1bc_sb[:, 0:1])
                    bwt = int(bw[t])
                    nc.vector.tensor_scalar(
                        out=oh_all[:, int(oh_off[t]):int(oh_off[t]) + bwt * WIN],
                        in0=iota_sb[:, :bwt * WIN],
                        scalar1=dstr_sb[:, t:t + 1],
                        scalar2=None, op0=mybir.AluOpType.is_equal)
                    w_sb = su_sb.tile([128, NW], BF16, name="w_sb")
                    for j in range(4):
                        o0 = j * 512
                        pw = su_pw.tile([128, 512], F32, name="pw")
                        if j < 3:
                            # bias added on DVE together with the bf16 cast
                            nc.tensor.matmul(pw[:, :nsz[j]], lhsT=h_t,
                                             rhs=en2wp_sb[:, o0:o0 + nsz[j]],
                                             start=True, stop=True)
                            nc.vector.tensor_tensor(
                                out=w_sb[:, o0:o0 + nsz[j]], in0=pw[:, :nsz[j]],
                                in1=bias_sb[:, o0:o0 + nsz[j]],
                                op=mybir.AluOpType.add)
                        else:
                            # bias via K=1 matmul; cast on Scalar
                            nc.tensor.matmul(pw[:, :nsz[j]], lhsT=onesbf_sb[:1, :],
                                             rhs=en2bp_sb[:, o0:o0 + nsz[j]],
                                             start=True, stop=False)
                            nc.tensor.matmul(pw[:, :nsz[j]], lhsT=h_t,
                                             rhs=en2wp_sb[:, o0:o0 + nsz[j]],
                                             start=False, stop=True)
                            nc.scalar.copy(w_sb[:, o0:o0 + nsz[j]],
                                           pw[:, :nsz[j]])
                    nc.sync.dma_start(w_dram[t * 128:(t + 1) * 128, :], w_sb[:])

            # =========== step pools ===========
            with (
                tc.tile_pool(name="st_w", bufs=6) as p_w,
                tc.tile_pool(name="st_x", bufs=12) as p_x,
                tc.tile_pool(name="st_m", bufs=4) as p_m,
                tc.tile_pool(name="st_sm", bufs=4) as p_sm,
                tc.tile_pool(name="ps_ag", bufs=3, space="PSUM") as ps_ag,
                tc.tile_pool(name="ps_up", bufs=2, space="PSUM") as ps_up,
                tc.tile_pool(name="ps_tr", bufs=1, space="PSUM") as ps_tr,
            ):
                def window_cols(w):
                    n0 = w * WIN
                    m = min(WIN, NPC - n0)
                    return n0, m

                def chunk_of(w):
                    return (w >= WB[1]) + (w >= WB[2])

                def update_window(w, outT_cur, outT_new, aggr_ps, step):
                    """Window epilogue: finish aggr, relu, update matmul, transpose, DMA."""
                    n0, m = window_cols(w)
                    last = step == STEPS
                    # + out (identity residual into conv); conv bias folds into relu
                    nc.tensor.matmul(aggr_ps[:, :m], lhsT=id_sb[:],
                                     rhs=outT_cur[:, n0:n0 + m], start=False, stop=True)
                    mT_sb = p_sm.tile([D_H, WIN], F32, name="mT_sb")
                    nc.scalar.activation(mT_sb[:, :m], aggr_ps[:, :m],
                                         mybir.ActivationFunctionType.Relu,
                                         bias=cvbc_sb[:, 0:1])
                    up = ps_up.tile([D_H, WIN], F32, name="up")
                    nc.tensor.matmul(up[:, :m], lhsT=mwt_sb[:], rhs=mT_sb[:, :m],
                                     start=True, stop=False)
                    nc.tensor.matmul(up[:, :m], lhsT=mwb_sb[:], rhs=outT_cur[:, n0:n0 + m],
                                     start=False, stop=not last)
                    if last:
                        nc.tensor.matmul(up[:, :m], lhsT=id_sb[:], rhs=nfT_sb[:, n0:n0 + m],
                                         start=False, stop=True)
                    # msg bias folds into the PSUM->SBUF copy
                    nc.scalar.activation(outT_new[:, n0:n0 + m], up[:, :m],
                                         mybir.ActivationFunctionType.Identity,
                                         bias=mbc_sb[:, 0:1])
                    tr = ps_tr.tile([128, D_H], F32, name="tr")
                    nc.tensor.transpose(tr[:m, :], outT_new[:, n0:n0 + m], id_sb[:])
                    if last:
                        rows = p_sm.tile([128, D_H], F32, name="rows_f")
                        nc.scalar.copy(rows[:m, :], tr[:m, :])
                        nc.sync.dma_start(y[n0:n0 + m, :], rows[:m, :])
                    else:
                        rows = p_sm.tile([128, D_H], BF16, name="rows_b")
                        nc.scalar.copy(rows[:m, :], tr[:m, :])
                        nc.sync.dma_start(cc_in[step % 2][n0:n0 + m, :],
                                          rows[:m, :])

                def all_gather(step):
                    nc.gpsimd.collective_compute(
                        "AllGather", mybir.AluOpType.bypass,
                        replica_groups=[list(range(N_CORES))],
                        ins=[cc_in[step % 2].opt()], outs=[cc_out[step].opt()])

                # =========== lin0: out0 = relu(n_feat @ lin0_w + b) ===========
                for w in range(N_WIN):
                    n0, m = window_cols(w)
                    up = ps_up.tile([D_H, WIN], F32, name="up")
                    nc.tensor.matmul(up[:, :m], lhsT=lin0w_sb[:], rhs=nfT_sb[:, n0:n0 + m],
                                     start=True, stop=True)
                    nc.scalar.activation(outT_a[:, n0:n0 + m], up[:, :m],
                                         mybir.ActivationFunctionType.Relu,
                                         bias=lin0bc_sb[:, 0:1])
                    tr = ps_tr.tile([128, D_H], F32, name="tr")
                    nc.tensor.transpose(tr[:m, :], outT_a[:, n0:n0 + m], id_sb[:])
                    rows = p_sm.tile([128, D_H], BF16, name="rows_b")
                    nc.scalar.copy(rows[:m, :], tr[:m, :])
                    nc.sync.dma_start(cc_in[0][n0:n0 + m, :], rows[:m, :])
                all_gather(0)

                # =========== message passing steps ===========
                for step in range(1, STEPS + 1):
                    outT_cur = outT_a if step % 2 == 1 else outT_b
                    outT_new = outT_b if step % 2 == 1 else outT_a
                    src_buf = cc_out[step - 1]
                    aggr_of = {}
                    for t in range(T):
                        x_g = p_x.tile([128, D_H], BF16, name="x_g")
                        nc.gpsimd.indirect_dma_start(
                            out=x_g[:], out_offset=None, in_=src_buf[:],
                            in_offset=bass.IndirectOffsetOnAxis(
                                ap=srci_sb[:, t:t + 1], axis=0))
                        w_t = p_w.tile([128, NW], BF16, name="w_t")
                        nc.sync.dma_start(w_t[:], w_dram[t * 128:(t + 1) * 128, :])
                        nc.vector._custom_dve(
                            PREFIX_MAC, out=pfx[:, 1:1 + NW], in0=w_t[:],
                            in1=x_g[:, None, :].to_broadcast([128, D_H, D_H]))
                        msg = p_m.tile([128, D_H], BF16, name="msg")
                        nc.vector.tensor_tensor(
                            out=msg[:], in0=pfx[:, D_H:1 + NW:D_H],
                            in1=pfx[:, 0:NW:D_H], op=mybir.AluOpType.subtract)
                        bwt = int(bw[t])
                        # scatter matmuls against the resident one-hot bank
                        for j in range(bwt):
                            w = int(w0[t]) + j
                            if w >= N_WIN:
                                continue
                            tiles_w = win_tiles[w]
                            if w not in aggr_of:
                                aggr_of[w] = ps_ag.tile([D_H, WIN], F32, name="aggr")
                            first = t == tiles_w[0]
                            last_t = t == tiles_w[-1]
                            o_c = int(oh_off[t]) + j * WIN
                            nc.tensor.matmul(aggr_of[w][:], lhsT=msg[:],
                                             rhs=oh_all[:, o_c:o_c + WIN],
                                             start=first, stop=False)
                            if last_t:
                                update_window(w, outT_cur, outT_new,
                                              aggr_of.pop(w), step)
                    if step < STEPS:
                        all_gather(step)

    nc.compile()
    return nc


_CACHED = {}


def kernel(n_feat, e_feat, src, dst, lin0_w, lin0_b, en1_w, en1_b,
           en2_w, en2_b, conv_bias, msg_w, msg_b):
    n_feat = np.asarray(n_feat, dtype=np.float32)
    e_feat = np.asarray(e_feat, dtype=np.float32)
    src = np.asarray(src, dtype=np.int32)
    dst = np.asarray(dst, dtype=np.int32)

    grid, per_core = _host_prep(n_feat, e_feat, src, dst)

    key = (grid["T"], grid["B_W"], tuple(grid["w0"].tolist()))
    if key not in _CACHED:
        _CACHED.clear()
        _CACHED[key] = _build_program(grid)
    nc = _CACHED[key]

    en2_wp = np.ascontiguousarray(
        np.asarray(en2_w, np.float32).reshape(E_H, D_H, D_H).transpose(0, 2, 1).reshape(E_H, NW))
    shared = dict(
        iota=np.tile(np.arange(grid["B_W"] * WIN, dtype=np.float32), (128, 1)).astype(BF),
        en1_w=np.asarray(en1_w, np.float32).astype(BF),
        en1_bc=np.asarray(en1_b, np.float32).reshape(E_H, 1),
        en2_wp=en2_wp.astype(BF),
        en2_bp=np.ascontiguousarray(
            np.asarray(en2_b, np.float32).reshape(D_H, D_H).T.reshape(1, NW)).astype(BF),
        lin0_wt=np.asarray(lin0_w, np.float32),
        lin0_bc=np.asarray(lin0_b, np.float32).reshape(D_H, 1),
        msgw_top=np.ascontiguousarray(np.asarray(msg_w, np.float32)[:D_H, :]),
        msgw_bot=np.ascontiguousarray(np.asarray(msg_w, np.float32)[D_H:, :]),
        msgb_c=np.asarray(msg_b, np.float32).reshape(D_H, 1),
        convb_c=np.asarray(conv_bias, np.float32).reshape(D_H, 1),
        ident=np.eye(D_H, dtype=np.float32),
        ones_bf=np.ones((1, 128), dtype=BF),
    )
    in_maps = []
    for k in range(N_CORES):
        m = dict(shared)
        m.update(per_core[k])
        in_maps.append(m)

    res = bass_utils.run_bass_kernel_spmd(nc, in_maps, core_ids=list(range(N_CORES)))
    out = np.concatenate([res.results[k]["y"] for k in range(N_CORES)], axis=0)
    return out.astype(np.float32)
